# revision 1
# baseline (speedup 1.0000x reference)
"""Trainium2 Bass kernel for nn_Baseline_9904194584728.

Pipeline: embedding gathers + MLP (293->64->64->64->9) + pnerf scan.

Key ideas:
  * Fold W0 into the embedding tables: KW = kmer_embed @ W0[16:272]
    (10648x64), SW = seq_embed @ W0[:16] + b0 (20x64). Tables stored as
    bf16 hi|lo pairs packed in 128-wide rows (256B) so dma_gather's
    transpose mode lands them directly in [feature, batch] layout;
    a stacked [I64;I64] identity matmul reconstitutes hi+lo into fp32
    PSUM exactly.
  * pnerf is algebraically an associative prefix product of rigid
    transforms: R_{i+1} = R_i M_i, c_{i+1} = c_i + R_i t_i where
    M_i = [ct^, n^ x ct^, n^], n^ = normalize(e1 x ct), t_i = ct_i.
    The device scan does: pointwise M build -> 24-step within-chunk
    prefix over 128 chunks (batched on partitions) -> hierarchical
    chunk-carry prefix -> batched apply of boundary transforms.
  * Data-parallel over B across the 8 cores (B_s = 32 per core).
"""

import sys
sys.path.insert(0, "/opt/trn_rl_repo")

import os
import numpy as np
import ml_dtypes
from contextlib import ExitStack

import concourse.bass as bass
import concourse.tile as tile
from concourse import bacc, mybir
from concourse.bass_utils import run_bass_kernel_spmd

F32 = mybir.dt.float32
BF16 = mybir.dt.bfloat16
I16 = mybir.dt.int16
AL = mybir.AluOpType
AF = mybir.ActivationFunctionType

NCORE = 8
L = 1024
B = 256
BS = B // NCORE            # 32 batch per core
TOK = L * BS               # 32768 tokens per core
NT = TOK // 512            # 64 batch-tiles of 512
NSUP = 8                   # supertiles of 4096 tokens (gather granularity)
NKMER = 10648
N3 = 3 * L                 # 3072 chain length
S = 24                     # chunk size (level-1)
C = N3 // S                # 128 chunks
EPS2 = 1e-24
KPH = os.environ.get("KPH", "ABC")


# --------------------------------------------------------------------------
# device kernel builder
# --------------------------------------------------------------------------

def _compose_views(t_ap, mode):
    """Return (pcol, arow, outv, col3) view factories for a [128, 384]
    transform tile.

    mode 'mj':  free = m*32 + lane   (m-major; lane = j or ch, 32 lanes)
    mode 'lm':  free = lane*12 + m   (lane-major)
    All views have dims (b, a, lane) with counts (4, 3, 32).
    """
    if mode == 'mj':
        def pcol(cc):
            v = t_ap[:, 3 * cc * 32:(3 * cc + 3) * 32]
            v = v.rearrange("p (a j) -> p a j", a=3)
            return v.unsqueeze(1).broadcast_to([128, 4, 3, 32])

        def arow(cc):
            v = t_ap[:, 0:384].rearrange("p (b three j) -> p b three j",
                                         b=4, three=3)
            v = v[:, :, cc, :]
            return v.unsqueeze(2).broadcast_to([128, 4, 3, 32])

        def outv():
            return t_ap[:, 0:384].rearrange("p (b a j) -> p b a j", b=4, a=3)

        def col3():
            return t_ap[:, 288:384]
    else:  # 'lm'
        def pcol(cc):
            v = t_ap[:, 0:384].rearrange("p (lan m) -> p lan m", lan=32)
            v = v[:, :, 3 * cc:3 * cc + 3]          # [p, lan, a]
            v = v.transpose([0, 2, 1])              # [p, a, lan]
            return v.unsqueeze(1).broadcast_to([128, 4, 3, 32])

        def arow(cc):
            v = t_ap[:, 0:384].rearrange("p (lan b three) -> p lan b three",
                                         lan=32, b=4)
            v = v[:, :, :, cc]                      # [p, lan, b]
            v = v.transpose([0, 2, 1])              # [p, b, lan]
            return v.unsqueeze(2).broadcast_to([128, 4, 3, 32])

        def outv():
            v = t_ap[:, 0:384].rearrange("p (lan b a) -> p lan b a",
                                         lan=32, b=4)
            return v.transpose([0, 2, 3, 1])        # [p, b, a, lan]

        def col3():
            v = t_ap[:, 0:384].rearrange("p (lan m) -> p lan m", lan=32)
            return v[:, :, 9:12]                    # [p, lan, a]
    return pcol, arow, outv, col3


def _emit_compose(nc, dst, P, A, tmpM, tmp2, mode):
    """dst = P o A for transform tiles [128, 384] in the given layout.
    tmpM/tmp2 are scratch [128, 384] tiles (same layout assumed; only
    used through the same view factories)."""
    Pp, _, _, Pc3 = _compose_views(P, mode)
    _, Aa, _, _ = _compose_views(A, mode)
    _, _, Mo, _ = _compose_views(tmpM, mode)
    _, _, To, _ = _compose_views(tmp2, mode)
    Dp, _, Do, Dc3 = _compose_views(dst, mode)
    nc.vector.tensor_tensor(Mo(), Pp(0), Aa(0), AL.mult)
    nc.vector.tensor_tensor(To(), Pp(1), Aa(1), AL.mult)
    nc.vector.tensor_tensor(tmpM[:, 0:384], tmpM[:, 0:384], tmp2[:, 0:384],
                            AL.add)
    nc.vector.tensor_tensor(To(), Pp(2), Aa(2), AL.mult)
    nc.vector.tensor_tensor(dst[:, 0:384], tmpM[:, 0:384], tmp2[:, 0:384],
                            AL.add)
    # translation: dst.t += P.t
    nc.vector.tensor_tensor(Dc3(), Dc3(), Pc3(), AL.add)


def build_nc():
    nc = bacc.Bacc("TRN2", target_bir_lowering=False, debug=False,
                   num_devices=NCORE)

    # ---------------- I/O ----------------
    d_ket = nc.declare_dram_parameter("ket", [256, NKMER], F32, isOutput=False)
    d_w0k = nc.declare_dram_parameter("w0k", [256, 64], F32, isOutput=False)
    d_w0p4 = nc.declare_dram_parameter("w0p4", [128, 64], F32, isOutput=False)
    d_swet = nc.declare_dram_parameter("swet", [16, 20], F32, isOutput=False)
    d_w0s = nc.declare_dram_parameter("w0s", [16, 64], F32, isOutput=False)
    d_b0row = nc.declare_dram_parameter("b0row", [1, 64], F32, isOutput=False)
    d_we = nc.declare_dram_parameter("wwe", [64, 64], F32, isOutput=False)
    d_w1 = nc.declare_dram_parameter("ww1", [64, 9], F32, isOutput=False)
    d_be = nc.declare_dram_parameter("becol", [64, 1], F32, isOutput=False)
    d_b1 = nc.declare_dram_parameter("b1col", [9, 1], F32, isOutput=False)
    d_idk = nc.declare_dram_parameter("identk", [128, 64], BF16, isOutput=False)
    d_idtf = nc.declare_dram_parameter("identtf", [128, 384], F32,
                                       isOutput=False)
    d_kidx = nc.declare_dram_parameter("kidx", [128, TOK // 16], I16,
                                       isOutput=False)
    d_sidx = nc.declare_dram_parameter("sidx", [128, TOK // 16], I16,
                                       isOutput=False)
    d_pssm = nc.declare_dram_parameter("pssm_pack", [128, 8192], F32,
                                       isOutput=False)
    o_scan = nc.declare_dram_parameter("o_scan", [128, 2304], F32,
                                       isOutput=True)

    # ---------------- internal DRAM ----------------
    kwp = nc.dram_tensor("kwp", [NKMER, 128], BF16)
    swp = nc.dram_tensor("swp", [20, 128], BF16)
    srf_d = nc.dram_tensor("srf_d", [9, TOK], F32)
    d_tc2 = nc.dram_tensor("d_tc2", [128, 384], F32)
    d_g = nc.dram_tensor("d_g", [128, 12], F32)
    d_b2 = nc.dram_tensor("d_b2", [128, 384], F32)

    with ExitStack() as ctx:
        tc = ctx.enter_context(tile.TileContext(nc))

        # persistent pool
        pw = ctx.enter_context(tc.tile_pool(name="pw", bufs=1))
        t_w0k = pw.tile([128, 128], F32, tag="w0k")     # two K-chunks side by side
        t_w0p4 = pw.tile([128, 64], F32, tag="w0p4")
        t_we = pw.tile([64, 64], F32, tag="we")
        t_w1 = pw.tile([64, 9], F32, tag="w1")
        t_be = pw.tile([64, 1], F32, tag="be")
        t_b1 = pw.tile([9, 1], F32, tag="b1")
        t_idk = pw.tile([128, 64], BF16, tag="idk")
        t_kidx = pw.tile([128, TOK // 16], I16, tag="kidx")
        t_sidx = pw.tile([128, TOK // 16], I16, tag="sidx")
        t_pssm = pw.tile([128, 8192], F32, tag="pssm")

        nc.sync.dma_start(t_w0k[:, 0:64], d_w0k[0:128, :])
        nc.sync.dma_start(t_w0k[:, 64:128], d_w0k[128:256, :])
        nc.sync.dma_start(t_w0p4[:], d_w0p4[:, :])
        nc.sync.dma_start(t_we[:], d_we[:, :])
        nc.sync.dma_start(t_w1[:], d_w1[:, :])
        nc.sync.dma_start(t_be[:], d_be[:, :])
        nc.sync.dma_start(t_b1[:], d_b1[:, :])
        nc.sync.dma_start(t_idk[:], d_idk[:, :])
        nc.sync.dma_start(t_kidx[:], d_kidx[:, :])
        nc.sync.dma_start(t_sidx[:], d_sidx[:, :])
        nc.sync.dma_start(t_pssm[:], d_pssm[:, :])

        # ---------------- phase A: tables ----------------
        with ExitStack() as actx:
            apool = actx.enter_context(tc.tile_pool(name="ap", bufs=2))
            apsum = actx.enter_context(
                tc.tile_pool(name="aps", bufs=2, space="PSUM"))
            a1 = actx.enter_context(tc.tile_pool(name="a1", bufs=1))

            # SW table: (20,64) = swet.T @ w0s + b0
            t_swet = a1.tile([16, 20], F32)
            t_w0s = a1.tile([16, 64], F32)
            t_b0r = a1.tile([20, 64], F32)
            nc.sync.dma_start(t_swet[:], d_swet[:, :])
            nc.sync.dma_start(t_w0s[:], d_w0s[:, :])
            nc.sync.dma_start(t_b0r[:], d_b0row[0:1, :].broadcast_to([20, 64]))
            ps_sw = apsum.tile([20, 64], F32)
            nc.tensor.matmul(ps_sw[:], t_swet[:], t_w0s[:], start=True,
                             stop=True)
            t_swf = a1.tile([20, 64], F32)
            nc.vector.tensor_tensor(t_swf[:], ps_sw[:], t_b0r[:], AL.add)
            t_swpk = a1.tile([20, 128], BF16)
            nc.scalar.activation(t_swpk[:, 0:64], t_swf[:], AF.Copy)
            nc.vector.tensor_tensor(t_swpk[:, 64:128], t_swf[:],
                                    t_swpk[:, 0:64], AL.subtract)
            nc.sync.dma_start(swp[:, :], t_swpk[:])

            # KW table: 84 row-tiles of 128 (last 24), streamed in 2048-row
            # column groups of ket
            NROW_GRP = 2048
            ngrp = ((NKMER + NROW_GRP - 1) // NROW_GRP) if "A" in KPH else 0
            for g in range(ngrp):
                r0 = g * NROW_GRP
                rn = min(NROW_GRP, NKMER - r0)
                kt0 = apool.tile([128, NROW_GRP], F32, tag="kt0")
                kt1 = apool.tile([128, NROW_GRP], F32, tag="kt1")
                nc.scalar.dma_start(kt0[:, 0:rn], d_ket[0:128, r0:r0 + rn])
                nc.scalar.dma_start(kt1[:, 0:rn], d_ket[128:256, r0:r0 + rn])
                for q0 in range(0, rn, 512):        # flush group of <=512 rows
                    qn = min(512, rn - q0)
                    kp = apool.tile([128, 512], BF16, tag="kp")
                    nsub = (qn + 127) // 128
                    for m in range(nsub):
                        rr0 = q0 + m * 128
                        rrn = min(128, qn - m * 128)
                        pk = apsum.tile([128, 64], F32, tag="pkw")
                        nc.tensor.matmul(pk[0:rrn, :], kt0[:, rr0:rr0 + rrn],
                                         t_w0k[:, 0:64], start=True, stop=False)
                        nc.tensor.matmul(pk[0:rrn, :], kt1[:, rr0:rr0 + rrn],
                                         t_w0k[:, 64:128], start=False,
                                         stop=True)
                        ksl = kp[0:rrn, 128 * m:128 * m + 128]
                        nc.scalar.activation(ksl[:, 0:64], pk[0:rrn, :],
                                             AF.Copy)
                        nc.vector.tensor_tensor(ksl[:, 64:128], pk[0:rrn, :],
                                                ksl[:, 0:64], AL.subtract)
                    dst = kwp[r0 + q0:r0 + q0 + qn, :]
                    if qn == 512:
                        dst = dst.rearrange("(s p) m -> p s m", s=4)
                        nc.sync.dma_start(
                            dst, kp[:].rearrange("p (s m) -> p s m", s=4))
                    else:  # partial tail group
                        for m in range((qn + 127) // 128):
                            rrn = min(128, qn - m * 128)
                            nc.sync.dma_start(
                                dst[128 * m:128 * m + rrn, :],
                                kp[0:rrn, 128 * m:128 * m + 128])

        # ---------------- phase B: MLP ----------------
        with ExitStack() as bctx:
            gp = bctx.enter_context(tc.tile_pool(name="gp", bufs=2))
            hb = bctx.enter_context(tc.tile_pool(name="hb", bufs=3))
            bps = bctx.enter_context(
                tc.tile_pool(name="bps", bufs=3, space="PSUM"))
            sps = bctx.enter_context(
                tc.tile_pool(name="sps", bufs=2, space="PSUM"))
            sf = bctx.enter_context(tc.tile_pool(name="sf", bufs=2))

            GW = TOK // NSUP                     # 4096 idx per gather
            for sup in range(NSUP if "B" in KPH else 0):
                kg = gp.tile([128, GW], BF16, tag="kg")
                sg = gp.tile([128, GW], BF16, tag="sg")
                isl = slice(sup * (GW // 16), (sup + 1) * (GW // 16))
                nc.gpsimd.dma_gather(
                    kg[:].rearrange("p (one n) -> p one n", one=1),
                    kwp[:, :], t_kidx[:, isl], num_idxs=GW, num_idxs_reg=GW,
                    elem_size=128, transpose=True, single_packet=False)
                nc.gpsimd.dma_gather(
                    sg[:].rearrange("p (one n) -> p one n", one=1),
                    swp[:, :], t_sidx[:, isl], num_idxs=GW, num_idxs_reg=GW,
                    elem_size=128, transpose=True, single_packet=False)
                srfS = sf.tile([9, GW], F32, tag="srfS")
                for tp in range(NT // NSUP):     # 8 batch-tiles per supertile
                    t = sup * (NT // NSUP) + tp
                    q, r = t % 4, t // 4
                    csl = slice(tp * 512, (tp + 1) * 512)
                    ph0 = bps.tile([64, 512], F32, tag="ph")
                    nc.tensor.matmul(ph0[:], t_idk[:], kg[:, csl],
                                     start=True, stop=False)
                    nc.tensor.matmul(ph0[:], t_idk[:], sg[:, csl],
                                     start=False, stop=False)
                    nc.tensor.matmul(
                        ph0[:], t_w0p4[32 * q:32 * q + 21, :],
                        t_pssm[32 * q:32 * q + 21, 512 * r:512 * r + 512],
                        start=False, stop=True,
                        tile_position=(32 * q, 0))
                    h0 = hb.tile([64, 512], F32, tag="h0")
                    nc.scalar.activation(h0[:], ph0[:], AF.Copy)
                    ph1 = bps.tile([64, 512], F32, tag="ph")
                    nc.tensor.matmul(ph1[:], t_we[:], h0[:], start=True,
                                     stop=True)
                    h1 = hb.tile([64, 512], F32, tag="h1")
                    nc.vector.tensor_scalar(h1[:], ph1[:], t_be[:], 0.0,
                                            AL.add, AL.max)
                    ph2 = bps.tile([64, 512], F32, tag="ph")
                    nc.tensor.matmul(ph2[:], t_we[:], h1[:], start=True,
                                     stop=True)
                    h2 = hb.tile([64, 512], F32, tag="h2")
                    nc.scalar.activation(h2[:], ph2[:], AF.Relu, bias=t_be[:],
                                         scale=1.0)
                    ps3 = sps.tile([9, 512], F32, tag="ps3")
                    nc.tensor.matmul(ps3[:], t_w1[:], h2[:], start=True,
                                     stop=True)
                    nc.vector.tensor_scalar(srfS[:, csl], ps3[:], t_b1[:],
                                            None, AL.add)
                nc.sync.dma_start(srf_d[:, sup * GW:(sup + 1) * GW], srfS[:])

        # ---------------- phase C: scan ----------------
        cp = ctx.enter_context(tc.tile_pool(name="cp", bufs=1))
        ct_all = cp.tile([128, 2304], F32, tag="ct")
        A_all = cp.tile([128, 24 * 384], F32, tag="Aall")
        q_all = cp.tile([128, 2304], F32, tag="qall")
        p_all = cp.tile([128, 2304], F32, tag="pall")
        sq_all = cp.tile([128, 2304], F32, tag="sqall")
        tmp768a = cp.tile([128, 768], F32, tag="t768a")
        tmp768b = cp.tile([128, 768], F32, tag="t768b")
        n2t = cp.tile([128, 768], F32, tag="n2")
        n2ct = cp.tile([128, 768], F32, tag="n2c")
        rnt = cp.tile([128, 768], F32, tag="rn")
        rnct = cp.tile([128, 768], F32, tag="rnc")
        t_idtf = cp.tile([128, 384], F32, tag="idtf")
        nc.sync.dma_start(t_idtf[:], d_idtf[:, :])

        if "C" in KPH:
            # C0: permute srf -> ct_all [c, (k*3+x)*32 + j]
            srf_r = srf_d.ap().rearrange("(r x) (c k1 j) -> r x c k1 j",
                                         r=3, x=3, c=128, k1=8)
            ct_r = ct_all[:].rearrange("p (k1 k2 x j) -> p k1 k2 x j",
                                       k1=8, k2=3, x=3)
            for k2 in range(3):
                for x in range(3):
                    src = srf_r[k2, x]                       # [c, k1, j]
                    nc.sync.dma_start(ct_r[:, :, k2, x, :], src)

            # C1: pointwise transform build
            ctv4 = ct_all[:].rearrange("p (k x j) -> p k x j", k=24, x=3)
            sqv4 = sq_all[:].rearrange("p (k x j) -> p k j x", k=24, x=3)
            Af = A_all[:].rearrange("p (k m j) -> p k m j", k=24, m=12)
            n2v = n2t[:].rearrange("p (k j) -> p k j", k=24)
            n2cv = n2ct[:].rearrange("p (k j) -> p k j", k=24)
            rnv3 = rnt[:].rearrange("p (k j) -> p k j", k=24).unsqueeze(2) \
                         .broadcast_to([128, 24, 3, 32])
            rncv = rnct[:].rearrange("p (k j) -> p k j", k=24)

            def ctx_(x):
                return ctv4[:, :, x, :]

            nc.scalar.activation(sq_all[:], ct_all[:], AF.Square)
            nc.vector.tensor_reduce(n2v.unsqueeze(-1), sqv4, mybir.AxisListType.X,
                                    AL.add)
            nc.vector.tensor_reduce(n2cv.unsqueeze(-1), sqv4[:, :, :, 1:3],
                                    mybir.AxisListType.X, AL.add)
            nc.vector.tensor_scalar_max(n2t[:], n2t[:], EPS2)
            nc.vector.tensor_scalar_max(n2ct[:], n2ct[:], EPS2)
            nc.scalar.activation(tmp768a[:], n2t[:], AF.Sqrt)
            nc.scalar.activation(tmp768b[:], n2ct[:], AF.Sqrt)
            nc.vector.reciprocal_approx_accurate(rnt[:], tmp768a[:], sq_all[:, 0:768])
            nc.vector.reciprocal_approx_accurate(rnct[:], tmp768b[:],
                                                 sq_all[:, 768:1536])
            # A columns: c0 = ct*rn ; t = ct ; c2 = (0, -z*rnc, y*rnc)
            nc.vector.tensor_tensor(Af[:, :, 0:3, :], ctv4, rnv3, AL.mult)
            nc.scalar.activation(Af[:, :, 9:12, :], ctv4, AF.Copy)
            nc.vector.tensor_scalar_mul(Af[:, :, 6, :], ctx_(0), 0.0)
            nc.vector.scalar_tensor_tensor(Af[:, :, 7, :], ctx_(2), -1.0, rncv,
                                           AL.mult, AL.mult)
            nc.vector.tensor_tensor(Af[:, :, 8, :], ctx_(1), rncv, AL.mult)
            # c1 = n^ x c0^
            nc.vector.tensor_tensor(Af[:, :, 3, :], Af[:, :, 7, :],
                                    Af[:, :, 2, :], AL.mult)
            nc.vector.tensor_tensor(tmp768a[:].rearrange("p (k j) -> p k j", k=24),
                                    Af[:, :, 8, :], Af[:, :, 1, :], AL.mult)
            nc.vector.tensor_tensor(Af[:, :, 3, :], Af[:, :, 3, :],
                                    tmp768a[:].rearrange("p (k j) -> p k j", k=24),
                                    AL.subtract)
            nc.vector.tensor_tensor(Af[:, :, 4, :], Af[:, :, 8, :],
                                    Af[:, :, 0, :], AL.mult)
            nc.vector.scalar_tensor_tensor(Af[:, :, 5, :], Af[:, :, 7, :], -1.0,
                                           Af[:, :, 0, :], AL.mult, AL.mult)

            # C2: level-1 scan (23 steps over k)
            Pa = cp.tile([128, 384], F32, tag="Pa")
            Pb = cp.tile([128, 384], F32, tag="Pb")
            tmpM = cp.tile([128, 384], F32, tag="tmpM")
            tmp2 = cp.tile([128, 384], F32, tag="tmp2")
            nc.scalar.activation(Pa[:], A_all[:, 0:384], AF.Copy)
            nc.scalar.activation(q_all[:, 0:96], A_all[:, 288:384], AF.Copy)
            cur, nxt = Pa, Pb
            for k in range(1, S):
                Ak = A_all[:, k * 384:(k + 1) * 384]
                _emit_compose(nc, nxt, cur, Ak, tmpM, tmp2, 'mj')
                nc.scalar.activation(q_all[:, k * 96:(k + 1) * 96],
                                     nxt[:, 288:384], AF.Copy)
                cur, nxt = nxt, cur
            Pfin = cur

            # C3: level-2 (chunk-carry exclusive prefix)
            # chunk c = 32*cl + ch; level-2 lanes: partition p = j + 32*cl,
            # free lanes ch (32), so all partition slices stay contiguous.
            # repack [c, m*32+j] -> [c, j*12+m] and bounce
            Palt = cp.tile([128, 384], F32, tag="Palt")
            nc.vector.tensor_copy(
                Palt[:].rearrange("p (j m) -> p j m", j=32),
                Pfin[:].rearrange("p (m j) -> p m j", m=12).transpose([0, 2, 1]))
            nc.sync.dma_start(d_tc2[:, :], Palt[:])
            T2 = cp.tile([128, 384], F32, tag="T2")
            tc2r = d_tc2.ap().rearrange("c (j m) -> c j m", j=32)
            for cl in range(4):
                # dst partitions j (block cl) <- rows c = 32*cl + ch
                src = tc2r[32 * cl:32 * cl + 32].transpose([1, 0, 2])  # [j, ch, m]
                nc.sync.dma_start(
                    T2[32 * cl:32 * cl + 32, :]
                    .rearrange("p (ch m) -> p ch m", ch=32), src)

            # inclusive hierarchical scan over ch (4 blocks x 8) on T2
            chS = cp.tile([128, 384], F32, tag="chS")
            nc.vector.tensor_copy(chS[:], T2[:])

            def lane_views(t_ap, lanes):
                """views for compose on lane-slices of an 'lm' tile; lanes is a
                list/slice spec (lo, n, step) on the 32 lanes."""
                lo, n, step = lanes
                base = t_ap[:, 0:384].rearrange("p (lan m) -> p lan m", lan=32)
                idx = base[:, lo:lo + (n - 1) * step + 1:step, :] if step > 1 \
                    else base[:, lo:lo + n, :]
                return idx  # [p, n, 12]

            def compose_lanes(dst_l, P_l, A_l, nl):
                """compose on [p, nl, 12] lane views (dims b,a,lane)."""
                def mk(v):
                    pc = v[:, :, 0:9].rearrange("p n (c a) -> p n c a", c=3)

                    def pcol(cc):
                        return pc[:, :, cc, :].transpose([0, 2, 1]) \
                            .unsqueeze(1).broadcast_to([128, 4, 3, nl])

                    ar = v.rearrange("p n (b three) -> p n b three", b=4)

                    def arow(cc):
                        return ar[:, :, :, cc].transpose([0, 2, 1]) \
                            .unsqueeze(2).broadcast_to([128, 4, 3, nl])

                    def outv():
                        return v.rearrange("p n (b a) -> p b a n", b=4)

                    def col3():
                        return v[:, :, 9:12]
                    return pcol, arow, outv, col3

                Pp, _, _, Pc3 = mk(P_l)
                _, Aa, _, _ = mk(A_l)
                tM = lane_views(tmpM, (0, nl, 1))
                t2 = lane_views(tmp2, (0, nl, 1))
                _, _, Mo, _ = mk(tM)
                _, _, To, _ = mk(t2)
                _, _, Do, Dc3 = mk(dst_l)
                nc.vector.tensor_tensor(Mo(), Pp(0), Aa(0), AL.mult)
                nc.vector.tensor_tensor(To(), Pp(1), Aa(1), AL.mult)
                nc.vector.tensor_tensor(Mo(), Mo(), To(), AL.add)
                nc.vector.tensor_tensor(To(), Pp(2), Aa(2), AL.mult)
                nc.vector.tensor_tensor(Do(), Mo(), To(), AL.add)
                nc.vector.tensor_tensor(Dc3(), Dc3(), Pc3(), AL.add)

            for w in range(1, 8):
                # lanes ch = blk*8 + w for blk 0..3
                prev = lane_views(chS, (w - 1, 4, 8))
                curA = lane_views(T2, (w, 4, 8))
                dst = lane_views(chS, (w, 4, 8))
                compose_lanes(dst, prev, curA, 4)

            btot = cp.tile([128, 48], F32, tag="btot")
            btv = btot[:].rearrange("p (n m) -> p n m", n=4)
            nc.vector.tensor_copy(btv[:, 0:1, :], lane_views(chS, (7, 1, 1)))
            for blk in range(1, 4):
                compose_lanes(btv[:, blk:blk + 1, :], btv[:, blk - 1:blk, :],
                              lane_views(chS, (blk * 8 + 7, 1, 1)), 1)

            Pchi = cp.tile([128, 384], F32, tag="Pchi")
            nc.vector.tensor_copy(Pchi[:, 0:96], chS[:, 0:96])
            # blocks 1..3: compose btot[blk-1] (bcast over w) with chS lanes
            for blk in range(1, 4):
                bview = btv[:, blk - 1:blk, :].broadcast_to([128, 8, 12])
                compose_lanes(lane_views(Pchi, (blk * 8, 8, 1)), bview,
                              lane_views(chS, (blk * 8, 8, 1)), 8)

            Pche = cp.tile([128, 384], F32, tag="Pche")
            nc.vector.tensor_copy(Pche[:, 0:12], t_idtf[:, 0:12])
            nc.vector.tensor_copy(Pche[:, 12:384], Pchi[:, 0:372])

            # cross-block (cl) exclusive prefix of block totals via DRAM bounce
            nc.sync.dma_start(d_g[:, :], Pchi[:, 372:384])
            G4 = cp.tile([128, 48], F32, tag="G4")
            for clp in range(4):
                src = d_g.ap()[32 * clp:32 * clp + 32, :]
                src = src.unsqueeze(0).broadcast_to([4, 32, 12])
                nc.sync.dma_start(G4[:, clp * 12:(clp + 1) * 12], src)
            g4v = G4[:].rearrange("p (n m) -> p n m", n=4)
            P01t = cp.tile([128, 12], F32, tag="P01t")
            P012t = cp.tile([128, 12], F32, tag="P012t")
            compose_lanes(P01t[:].unsqueeze(1), g4v[:, 0:1, :], g4v[:, 1:2, :], 1)
            compose_lanes(P012t[:].unsqueeze(1), P01t[:].unsqueeze(1),
                          g4v[:, 2:3, :], 1)
            Pexcl = cp.tile([128, 12], F32, tag="Pexcl")
            nc.vector.tensor_copy(Pexcl[0:32, :], t_idtf[0:32, 0:12])
            nc.vector.tensor_copy(Pexcl[32:64, :], G4[32:64, 0:12])
            nc.vector.tensor_copy(Pexcl[64:96, :], P01t[64:96, :])
            nc.vector.tensor_copy(Pexcl[96:128, :], P012t[96:128, :])

            # B_chunk (in level-2 lane layout) = Pexcl o S_excl
            Bcj = cp.tile([128, 384], F32, tag="Bcj")
            compose_lanes(lane_views(Bcj, (0, 32, 1)),
                          Pexcl[:].unsqueeze(1).broadcast_to([128, 32, 12]),
                          lane_views(Pche, (0, 32, 1)), 32)
            nc.sync.dma_start(d_b2[:, :], Bcj[:])
            Bch = cp.tile([128, 384], F32, tag="Bch")
            b2r = d_b2.ap().rearrange("p (ch m) -> p ch m", ch=32)
            for cl in range(4):
                src = b2r[32 * cl:32 * cl + 32].transpose([1, 0, 2])  # [ch, j, m]
                nc.sync.dma_start(
                    Bch[32 * cl:32 * cl + 32, :]
                    .rearrange("p (j m) -> p j m", j=32), src)

            # C4: apply  p = B.t + B.R @ q
            qv = q_all[:].rearrange("p (k x j) -> p k x j", k=24, x=3)
            Bv = Bch[:].rearrange("p (j m) -> p j m", j=32)
            pv = p_all[:].rearrange("p (k a j) -> p k a j", k=24, a=3)
            tA = sq_all[:]  # reuse as scratch [128, 2304]
            tAv = tA.rearrange("p (k a j) -> p k a j", k=24, a=3)
            tB = ct_all[:]  # reuse as scratch
            tBv = tB.rearrange("p (k a j) -> p k a j", k=24, a=3)

            def qx(cc):
                return qv[:, :, cc, :].unsqueeze(2).broadcast_to([128, 24, 3, 32])

            def bcol(cc):
                v = Bv[:, :, 3 * cc:3 * cc + 3].transpose([0, 2, 1])  # [p,a,j]
                return v.unsqueeze(1).broadcast_to([128, 24, 3, 32])

            nc.vector.tensor_tensor(tAv, qx(0), bcol(0), AL.mult)
            nc.vector.tensor_tensor(tBv, qx(1), bcol(1), AL.mult)
            nc.vector.tensor_tensor(tAv, tAv, tBv, AL.add)
            nc.vector.tensor_tensor(tBv, qx(2), bcol(2), AL.mult)
            nc.vector.tensor_tensor(tAv, tAv, tBv, AL.add)
            nc.vector.tensor_tensor(pv, tAv, bcol(3), AL.add)
            nc.sync.dma_start(o_scan[:, :], p_all[:])

    nc.compile()
    return nc


# --------------------------------------------------------------------------
# host wrapper
# --------------------------------------------------------------------------

_NC_CACHE = []


def _get_nc():
    if not _NC_CACHE:
        _NC_CACHE.append(build_nc())
    return _NC_CACHE[0]


def _wrap_idx(flat_idx):
    """int array (32768,) -> [128, 2048] int16 wrapped + replicated."""
    w = flat_idx.astype(np.int16).reshape(TOK // 16, 16).T  # [16, TOK/16]
    return np.tile(w, (8, 1)).copy()


def make_in_maps(inputs):
    seq = np.asarray(inputs["seq"])
    kmer = np.asarray(inputs["kmer"])
    pssm = np.asarray(inputs["pssm"], dtype=np.float32)
    seq_embed = np.asarray(inputs["seq_embed"], dtype=np.float32)
    kmer_embed = np.asarray(inputs["kmer_embed"], dtype=np.float32)
    W0 = np.asarray(inputs["W0"], dtype=np.float32)
    b0 = np.asarray(inputs["b0"], dtype=np.float32)
    We = np.asarray(inputs["We"], dtype=np.float32)
    be = np.asarray(inputs["be"], dtype=np.float32)
    W1 = np.asarray(inputs["W1"], dtype=np.float32)
    b1 = np.asarray(inputs["b1"], dtype=np.float32)

    # shared (replicated) tensors
    ket = np.ascontiguousarray(kmer_embed.T)                  # (256, 10648)
    w0k = np.ascontiguousarray(W0[16:272])                    # (256, 64)
    w0p4 = np.zeros((128, 64), np.float32)
    for q in range(4):
        w0p4[32 * q:32 * q + 21] = W0[272:293]
    swet = np.ascontiguousarray(seq_embed.T)                  # (16, 20)
    w0s = np.ascontiguousarray(W0[:16])                       # (16, 64)
    b0row = b0[None, :]
    becol = np.ascontiguousarray(be[:, None])
    b1col = np.ascontiguousarray(b1[:, None])
    identk = np.tile(np.eye(64, dtype=ml_dtypes.bfloat16), (2, 1))
    id12 = np.array([1, 0, 0, 0, 1, 0, 0, 0, 1, 0, 0, 0], np.float32)
    identtf = np.broadcast_to(np.tile(id12, 32), (128, 384)).copy()

    shared = dict(ket=ket, w0k=w0k, w0p4=w0p4, swet=swet, w0s=w0s,
                  b0row=b0row, wwe=We, ww1=np.ascontiguousarray(W1),
                  becol=becol, b1col=b1col, identk=identk, identtf=identtf)

    in_maps = []
    for c in range(NCORE):
        bsl = slice(c * BS, (c + 1) * BS)
        kidx = _wrap_idx(kmer[:, bsl].reshape(TOK))
        sidx = _wrap_idx(seq[:, bsl].reshape(TOK))
        pf = pssm[:, bsl, :].reshape(TOK, 21)                 # g = l*32+j
        arr = pf.reshape(16, 4, 512, 21)                      # r, q, i, f
        pack = np.zeros((128, 8192), np.float32)
        for q in range(4):
            pack[32 * q:32 * q + 21] = \
                arr[:, q].transpose(2, 0, 1).reshape(21, 8192)
        in_maps.append(dict(shared, kidx=kidx, sidx=sidx, pssm_pack=pack))
    return in_maps


def unpack_output(per_core_oscan):
    out = np.empty((N3, B, 3), np.float32)
    for c in range(NCORE):
        arr = np.asarray(per_core_oscan[c]).reshape(128, 24, 3, 32)
        out[:, c * BS:(c + 1) * BS, :] = \
            arr.transpose(0, 1, 3, 2).reshape(N3, BS, 3)
    return out


def kernel(**inputs):
    nc = _get_nc()
    in_maps = make_in_maps(inputs)
    res = run_bass_kernel_spmd(nc, in_maps, list(range(NCORE)))
    return unpack_output([res.results[c]["o_scan"] for c in range(NCORE)])



# revision 4
# speedup vs baseline: 4.7242x; 4.7242x over previous
"""Trainium2 Bass kernel for nn_Baseline_9904194584728.

Pipeline: embedding gathers + MLP (293->64->64->64->9) + pnerf scan.

The wall-clock of a call is dominated by the axon tunnel (~30-40 MB/s each
way), so the kernel is organized around minimizing bytes on the wire:

  * The kmer table is folded with W0 on the host (KW = kmer_embed @
    W0[16:272], 10648x64) and shipped as int16 fixed-point (scale 0.25),
    sharded 1/8 per core; an on-device AllGather rebuilds the full table,
    which is then dequantized and expanded into bf16 hi|lo pairs packed in
    128-wide rows (256B) so dma_gather's transpose mode lands them directly
    in [feature, batch] layout; a stacked [I64;I64] identity matmul
    reconstitutes hi+lo into fp32 PSUM.
  * pssm ships as uint16 fixed-point (biased int16), dequantized on the DVE.
  * Gather indices ship as [16, TOK/16] int16 and are replicated to the
    128-partition wrap layout by a broadcast DMA on device.
  * The output ships as fp16 (the pnerf scan amplifies *input* noise ~30x,
    but output quantization is post-scan and harmless).
  * A persistent jit of the shard_map'd bass_exec call avoids per-call
    retracing, and the output is fetched from device exactly once.

pnerf is algebraically an associative prefix product of rigid transforms:
R_{i+1} = R_i M_i, c_{i+1} = c_i + R_i t_i. The device scan does: pointwise
M build -> 24-step within-chunk prefix over 128 chunks (batched on
partitions) -> hierarchical chunk-carry prefix -> batched apply of boundary
transforms. Data-parallel over B across the 8 cores (B_s = 32 per core).
"""

import sys
sys.path.insert(0, "/opt/trn_rl_repo")

import os
import numpy as np
import ml_dtypes
from contextlib import ExitStack

import concourse.bass as bass
import concourse.tile as tile
from concourse import bacc, mybir

F32 = mybir.dt.float32
F16 = mybir.dt.float16
BF16 = mybir.dt.bfloat16
I16 = mybir.dt.int16
AL = mybir.AluOpType
AF = mybir.ActivationFunctionType

NCORE = 8
L = 1024
B = 256
BS = B // NCORE            # 32 batch per core
TOK = L * BS               # 32768 tokens per core
NT = TOK // 512            # 64 batch-tiles of 512
NSUP = 8                   # supertiles of 4096 tokens (gather granularity)
NKMER = 10648
NKPAD = 10752              # padded to 128*84 for the [128, 84*64] SBUF view
KSH = NKPAD // NCORE       # 1344 table rows shipped per core
N3 = 3 * L                 # 3072 chain length
S = 24                     # chunk size (level-1)
C = N3 // S                # 128 chunks
EPS2 = 1e-24
TSCALE = 0.25              # int16 table scale: value = q * TSCALE / 32767
COLL = os.environ.get("KCOLL", "1") == "1"


# --------------------------------------------------------------------------
# device kernel builder
# --------------------------------------------------------------------------

def _compose_views(t_ap, mode):
    """Return (pcol, arow, outv, col3) view factories for a [128, 384]
    transform tile.

    mode 'mj':  free = m*32 + lane   (m-major; lane = j or ch, 32 lanes)
    mode 'lm':  free = lane*12 + m   (lane-major)
    All views have dims (b, a, lane) with counts (4, 3, 32).
    """
    if mode == 'mj':
        def pcol(cc):
            v = t_ap[:, 3 * cc * 32:(3 * cc + 3) * 32]
            v = v.rearrange("p (a j) -> p a j", a=3)
            return v.unsqueeze(1).broadcast_to([128, 4, 3, 32])

        def arow(cc):
            v = t_ap[:, 0:384].rearrange("p (b three j) -> p b three j",
                                         b=4, three=3)
            v = v[:, :, cc, :]
            return v.unsqueeze(2).broadcast_to([128, 4, 3, 32])

        def outv():
            return t_ap[:, 0:384].rearrange("p (b a j) -> p b a j", b=4, a=3)

        def col3():
            return t_ap[:, 288:384]
    else:  # 'lm'
        def pcol(cc):
            v = t_ap[:, 0:384].rearrange("p (lan m) -> p lan m", lan=32)
            v = v[:, :, 3 * cc:3 * cc + 3]          # [p, lan, a]
            v = v.transpose([0, 2, 1])              # [p, a, lan]
            return v.unsqueeze(1).broadcast_to([128, 4, 3, 32])

        def arow(cc):
            v = t_ap[:, 0:384].rearrange("p (lan b three) -> p lan b three",
                                         lan=32, b=4)
            v = v[:, :, :, cc]                      # [p, lan, b]
            v = v.transpose([0, 2, 1])              # [p, b, lan]
            return v.unsqueeze(2).broadcast_to([128, 4, 3, 32])

        def outv():
            v = t_ap[:, 0:384].rearrange("p (lan b a) -> p lan b a",
                                         lan=32, b=4)
            return v.transpose([0, 2, 3, 1])        # [p, b, a, lan]

        def col3():
            v = t_ap[:, 0:384].rearrange("p (lan m) -> p lan m", lan=32)
            return v[:, :, 9:12]                    # [p, lan, a]
    return pcol, arow, outv, col3


def _emit_compose(nc, dst, P, A, tmpM, tmp2, mode):
    """dst = P o A for transform tiles [128, 384] in the given layout.
    tmpM/tmp2 are scratch [128, 384] tiles (same layout assumed; only
    used through the same view factories)."""
    Pp, _, _, Pc3 = _compose_views(P, mode)
    _, Aa, _, _ = _compose_views(A, mode)
    _, _, Mo, _ = _compose_views(tmpM, mode)
    _, _, To, _ = _compose_views(tmp2, mode)
    Dp, _, Do, Dc3 = _compose_views(dst, mode)
    nc.vector.tensor_tensor(Mo(), Pp(0), Aa(0), AL.mult)
    nc.vector.tensor_tensor(To(), Pp(1), Aa(1), AL.mult)
    nc.vector.tensor_tensor(tmpM[:, 0:384], tmpM[:, 0:384], tmp2[:, 0:384],
                            AL.add)
    nc.vector.tensor_tensor(To(), Pp(2), Aa(2), AL.mult)
    nc.vector.tensor_tensor(dst[:, 0:384], tmpM[:, 0:384], tmp2[:, 0:384],
                            AL.add)
    # translation: dst.t += P.t
    nc.vector.tensor_tensor(Dc3(), Dc3(), Pc3(), AL.add)


def build_nc():
    nc = bacc.Bacc("TRN2", target_bir_lowering=False, debug=False,
                   num_devices=NCORE)

    # ---------------- I/O ----------------
    kw_rows = KSH if COLL else NKPAD
    d_kwt = nc.declare_dram_parameter("kwt", [kw_rows, 64], I16, isOutput=False)
    d_swp = nc.declare_dram_parameter("swp", [20, 128], BF16, isOutput=False)
    d_w0p = nc.declare_dram_parameter("w0p", [21, 64], F32, isOutput=False)
    d_we = nc.declare_dram_parameter("wwe", [64, 64], F32, isOutput=False)
    d_w1 = nc.declare_dram_parameter("ww1", [64, 9], F32, isOutput=False)
    d_be = nc.declare_dram_parameter("becol", [64, 1], F32, isOutput=False)
    d_b1 = nc.declare_dram_parameter("b1col", [9, 1], F32, isOutput=False)
    d_idk = nc.declare_dram_parameter("identk", [128, 64], BF16, isOutput=False)
    d_id12 = nc.declare_dram_parameter("id12", [1, 12], F32, isOutput=False)
    d_kidx = nc.declare_dram_parameter("kidx", [16, TOK // 16], I16,
                                       isOutput=False)
    d_sidx = nc.declare_dram_parameter("sidx", [16, TOK // 16], I16,
                                       isOutput=False)
    d_pssm = nc.declare_dram_parameter("pssm_q", [84, 8192], I16,
                                       isOutput=False)
    o_scan = nc.declare_dram_parameter("o_scan", [128, 2304], F16,
                                       isOutput=True)

    # ---------------- internal DRAM ----------------
    kwp = nc.dram_tensor("kwp", [NKPAD, 128], BF16)
    srf_d = nc.dram_tensor("srf_d", [9, TOK], F32)
    d_tc2 = nc.dram_tensor("d_tc2", [128, 384], F32)
    d_g = nc.dram_tensor("d_g", [128, 12], F32)
    d_b2 = nc.dram_tensor("d_b2", [128, 384], F32)
    if COLL:
        cc_in = nc.dram_tensor("cc_in", [KSH, 64], I16)
        cc_out = nc.dram_tensor("cc_out", [NKPAD, 64], I16,
                                addr_space="Shared")

    with ExitStack() as ctx:
        tc = ctx.enter_context(tile.TileContext(nc))

        # persistent pool
        pw = ctx.enter_context(tc.tile_pool(name="pw", bufs=1))
        t_w0p4 = pw.tile([128, 64], F32, tag="w0p4")
        t_we = pw.tile([64, 64], F32, tag="we")
        t_w1 = pw.tile([64, 9], F32, tag="w1")
        t_be = pw.tile([64, 1], F32, tag="be")
        t_b1 = pw.tile([9, 1], F32, tag="b1")
        t_idk = pw.tile([128, 64], BF16, tag="idk")
        t_kidx = pw.tile([128, TOK // 16], I16, tag="kidx")
        t_sidx = pw.tile([128, TOK // 16], I16, tag="sidx")
        t_pssm = pw.tile([128, 8192], F32, tag="pssm")
        t_idtf = pw.tile([128, 12], F32, tag="idtf")

        for q in range(4):
            nc.sync.dma_start(t_w0p4[32 * q:32 * q + 21, :], d_w0p[:, :])
        nc.sync.dma_start(t_we[:], d_we[:, :])
        nc.sync.dma_start(t_w1[:], d_w1[:, :])
        nc.sync.dma_start(t_be[:], d_be[:, :])
        nc.sync.dma_start(t_b1[:], d_b1[:, :])
        nc.sync.dma_start(t_idk[:], d_idk[:, :])
        nc.sync.dma_start(t_idtf[:], d_id12[0:1, :].broadcast_to([128, 12]))
        # wrap-replicate the gather indices 16 -> 128 partitions
        for g in range(8):
            nc.sync.dma_start(t_kidx[16 * g:16 * g + 16, :], d_kidx[:, :])
            nc.sync.dma_start(t_sidx[16 * g:16 * g + 16, :], d_sidx[:, :])

        # ---------------- phase A: table expand + pssm dequant ----------
        with ExitStack() as actx:
            ap_ = actx.enter_context(tc.tile_pool(name="ap", bufs=1))

            # pssm: biased-int16 -> fp32 in [0, 1]
            t_praw = ap_.tile([128, 8192], I16, tag="praw")
            for q in range(4):
                nc.sync.dma_start(t_praw[32 * q:32 * q + 21, :],
                                  d_pssm[21 * q:21 * q + 21, :])
                nc.vector.tensor_scalar(
                    t_pssm[32 * q:32 * q + 21, :],
                    t_praw[32 * q:32 * q + 21, :],
                    32768.0, 1.0 / 65535.0, AL.add, AL.mult)

            # kmer table: int16 shard -> (AllGather) -> bf16 hi|lo rows
            if COLL:
                nc.sync.dma_start(cc_in[:, :], d_kwt[:, :])
                nc.gpsimd.collective_compute(
                    "AllGather", AL.bypass,
                    replica_groups=[list(range(NCORE))],
                    ins=[cc_in.ap().opt()],
                    outs=[cc_out.ap().opt()])
                src_tbl = cc_out.ap()
            else:
                src_tbl = d_kwt[:, :]
            t_raw = ap_.tile([128, 5376], I16, tag="traw")
            nc.sync.dma_start(
                t_raw[:].rearrange("p (q c) -> p q c", q=84),
                src_tbl.rearrange("(p q) c -> p q c", p=128))
            t_tf = ap_.tile([128, 5376], F32, tag="ttf")
            nc.vector.tensor_scalar_mul(t_tf[:], t_raw[:], TSCALE / 32767.0)
            t_hi = ap_.tile([128, 5376], BF16, tag="thi")
            nc.scalar.activation(t_hi[:], t_tf[:], AF.Copy)
            t_lo = ap_.tile([128, 5376], BF16, tag="tlo")
            nc.vector.tensor_tensor(t_lo[:], t_tf[:], t_hi[:], AL.subtract)
            kwv = kwp.ap().rearrange("(p q) (h c) -> p h q c", p=128, h=2)
            nc.sync.dma_start(kwv[:, 0],
                              t_hi[:].rearrange("p (q c) -> p q c", q=84))
            nc.sync.dma_start(kwv[:, 1],
                              t_lo[:].rearrange("p (q c) -> p q c", q=84))

        # ---------------- phase B: MLP ----------------
        with ExitStack() as bctx:
            gp = bctx.enter_context(tc.tile_pool(name="gp", bufs=2))
            hb = bctx.enter_context(tc.tile_pool(name="hb", bufs=3))
            bps = bctx.enter_context(
                tc.tile_pool(name="bps", bufs=3, space="PSUM"))
            sps = bctx.enter_context(
                tc.tile_pool(name="sps", bufs=2, space="PSUM"))
            sf = bctx.enter_context(tc.tile_pool(name="sf", bufs=2))

            GW = TOK // NSUP                     # 4096 idx per gather
            for sup in range(NSUP):
                kg = gp.tile([128, GW], BF16, tag="kg")
                sg = gp.tile([128, GW], BF16, tag="sg")
                isl = slice(sup * (GW // 16), (sup + 1) * (GW // 16))
                nc.gpsimd.dma_gather(
                    kg[:].rearrange("p (one n) -> p one n", one=1),
                    kwp[:, :], t_kidx[:, isl], num_idxs=GW, num_idxs_reg=GW,
                    elem_size=128, transpose=True, single_packet=False)
                nc.gpsimd.dma_gather(
                    sg[:].rearrange("p (one n) -> p one n", one=1),
                    d_swp[:, :], t_sidx[:, isl], num_idxs=GW, num_idxs_reg=GW,
                    elem_size=128, transpose=True, single_packet=False)
                srfS = sf.tile([9, GW], F32, tag="srfS")
                for tp in range(NT // NSUP):     # 8 batch-tiles per supertile
                    t = sup * (NT // NSUP) + tp
                    q, r = t % 4, t // 4
                    csl = slice(tp * 512, (tp + 1) * 512)
                    ph0 = bps.tile([64, 512], F32, tag="ph")
                    nc.tensor.matmul(ph0[:], t_idk[:], kg[:, csl],
                                     start=True, stop=False)
                    nc.tensor.matmul(ph0[:], t_idk[:], sg[:, csl],
                                     start=False, stop=False)
                    nc.tensor.matmul(
                        ph0[:], t_w0p4[32 * q:32 * q + 21, :],
                        t_pssm[32 * q:32 * q + 21, 512 * r:512 * r + 512],
                        start=False, stop=True,
                        tile_position=(32 * q, 0))
                    h0 = hb.tile([64, 512], F32, tag="h0")
                    nc.scalar.activation(h0[:], ph0[:], AF.Copy)
                    ph1 = bps.tile([64, 512], F32, tag="ph")
                    nc.tensor.matmul(ph1[:], t_we[:], h0[:], start=True,
                                     stop=True)
                    h1 = hb.tile([64, 512], F32, tag="h1")
                    nc.vector.tensor_scalar(h1[:], ph1[:], t_be[:], 0.0,
                                            AL.add, AL.max)
                    ph2 = bps.tile([64, 512], F32, tag="ph")
                    nc.tensor.matmul(ph2[:], t_we[:], h1[:], start=True,
                                     stop=True)
                    h2 = hb.tile([64, 512], F32, tag="h2")
                    nc.scalar.activation(h2[:], ph2[:], AF.Relu, bias=t_be[:],
                                         scale=1.0)
                    ps3 = sps.tile([9, 512], F32, tag="ps3")
                    nc.tensor.matmul(ps3[:], t_w1[:], h2[:], start=True,
                                     stop=True)
                    nc.vector.tensor_scalar(srfS[:, csl], ps3[:], t_b1[:],
                                            None, AL.add)
                nc.sync.dma_start(srf_d[:, sup * GW:(sup + 1) * GW], srfS[:])

        # ---------------- phase C: scan ----------------
        cp = ctx.enter_context(tc.tile_pool(name="cp", bufs=1))
        ct_all = cp.tile([128, 2304], F32, tag="ct")
        A_all = cp.tile([128, 24 * 384], F32, tag="Aall")
        q_all = cp.tile([128, 2304], F32, tag="qall")
        p_all = cp.tile([128, 2304], F32, tag="pall")
        sq_all = cp.tile([128, 2304], F32, tag="sqall")
        tmp768a = cp.tile([128, 768], F32, tag="t768a")
        tmp768b = cp.tile([128, 768], F32, tag="t768b")
        n2t = cp.tile([128, 768], F32, tag="n2")
        n2ct = cp.tile([128, 768], F32, tag="n2c")
        rnt = cp.tile([128, 768], F32, tag="rn")
        rnct = cp.tile([128, 768], F32, tag="rnc")

        # C0: permute srf -> ct_all [c, (k*3+x)*32 + j]
        srf_r = srf_d.ap().rearrange("(r x) (c k1 j) -> r x c k1 j",
                                     r=3, x=3, c=128, k1=8)
        ct_r = ct_all[:].rearrange("p (k1 k2 x j) -> p k1 k2 x j",
                                   k1=8, k2=3, x=3)
        for k2 in range(3):
            for x in range(3):
                src = srf_r[k2, x]                       # [c, k1, j]
                nc.sync.dma_start(ct_r[:, :, k2, x, :], src)

        # C1: pointwise transform build
        ctv4 = ct_all[:].rearrange("p (k x j) -> p k x j", k=24, x=3)
        sqv4 = sq_all[:].rearrange("p (k x j) -> p k j x", k=24, x=3)
        Af = A_all[:].rearrange("p (k m j) -> p k m j", k=24, m=12)
        n2v = n2t[:].rearrange("p (k j) -> p k j", k=24)
        n2cv = n2ct[:].rearrange("p (k j) -> p k j", k=24)
        rnv3 = rnt[:].rearrange("p (k j) -> p k j", k=24).unsqueeze(2) \
                     .broadcast_to([128, 24, 3, 32])
        rncv = rnct[:].rearrange("p (k j) -> p k j", k=24)

        def ctx_(x):
            return ctv4[:, :, x, :]

        nc.scalar.activation(sq_all[:], ct_all[:], AF.Square)
        nc.vector.tensor_reduce(n2v.unsqueeze(-1), sqv4, mybir.AxisListType.X,
                                AL.add)
        nc.vector.tensor_reduce(n2cv.unsqueeze(-1), sqv4[:, :, :, 1:3],
                                mybir.AxisListType.X, AL.add)
        nc.vector.tensor_scalar_max(n2t[:], n2t[:], EPS2)
        nc.vector.tensor_scalar_max(n2ct[:], n2ct[:], EPS2)
        nc.scalar.activation(tmp768a[:], n2t[:], AF.Sqrt)
        nc.scalar.activation(tmp768b[:], n2ct[:], AF.Sqrt)
        nc.vector.reciprocal_approx_accurate(rnt[:], tmp768a[:], sq_all[:, 0:768])
        nc.vector.reciprocal_approx_accurate(rnct[:], tmp768b[:],
                                             sq_all[:, 768:1536])
        # A columns: c0 = ct*rn ; t = ct ; c2 = (0, -z*rnc, y*rnc)
        nc.vector.tensor_tensor(Af[:, :, 0:3, :], ctv4, rnv3, AL.mult)
        nc.scalar.activation(Af[:, :, 9:12, :], ctv4, AF.Copy)
        nc.vector.tensor_scalar_mul(Af[:, :, 6, :], ctx_(0), 0.0)
        nc.vector.scalar_tensor_tensor(Af[:, :, 7, :], ctx_(2), -1.0, rncv,
                                       AL.mult, AL.mult)
        nc.vector.tensor_tensor(Af[:, :, 8, :], ctx_(1), rncv, AL.mult)
        # c1 = n^ x c0^
        nc.vector.tensor_tensor(Af[:, :, 3, :], Af[:, :, 7, :],
                                Af[:, :, 2, :], AL.mult)
        nc.vector.tensor_tensor(tmp768a[:].rearrange("p (k j) -> p k j", k=24),
                                Af[:, :, 8, :], Af[:, :, 1, :], AL.mult)
        nc.vector.tensor_tensor(Af[:, :, 3, :], Af[:, :, 3, :],
                                tmp768a[:].rearrange("p (k j) -> p k j", k=24),
                                AL.subtract)
        nc.vector.tensor_tensor(Af[:, :, 4, :], Af[:, :, 8, :],
                                Af[:, :, 0, :], AL.mult)
        nc.vector.scalar_tensor_tensor(Af[:, :, 5, :], Af[:, :, 7, :], -1.0,
                                       Af[:, :, 0, :], AL.mult, AL.mult)

        # C2: level-1 scan (23 steps over k)
        Pa = cp.tile([128, 384], F32, tag="Pa")
        Pb = cp.tile([128, 384], F32, tag="Pb")
        tmpM = cp.tile([128, 384], F32, tag="tmpM")
        tmp2 = cp.tile([128, 384], F32, tag="tmp2")
        nc.scalar.activation(Pa[:], A_all[:, 0:384], AF.Copy)
        nc.scalar.activation(q_all[:, 0:96], A_all[:, 288:384], AF.Copy)
        cur, nxt = Pa, Pb
        for k in range(1, S):
            Ak = A_all[:, k * 384:(k + 1) * 384]
            _emit_compose(nc, nxt, cur, Ak, tmpM, tmp2, 'mj')
            nc.scalar.activation(q_all[:, k * 96:(k + 1) * 96],
                                 nxt[:, 288:384], AF.Copy)
            cur, nxt = nxt, cur
        Pfin = cur

        # C3: level-2 (chunk-carry exclusive prefix)
        # chunk c = 32*cl + ch; level-2 lanes: partition p = j + 32*cl,
        # free lanes ch (32), so all partition slices stay contiguous.
        # repack [c, m*32+j] -> [c, j*12+m] and bounce
        Palt = cp.tile([128, 384], F32, tag="Palt")
        nc.vector.tensor_copy(
            Palt[:].rearrange("p (j m) -> p j m", j=32),
            Pfin[:].rearrange("p (m j) -> p m j", m=12).transpose([0, 2, 1]))
        nc.sync.dma_start(d_tc2[:, :], Palt[:])
        T2 = cp.tile([128, 384], F32, tag="T2")
        tc2r = d_tc2.ap().rearrange("c (j m) -> c j m", j=32)
        for cl in range(4):
            # dst partitions j (block cl) <- rows c = 32*cl + ch
            src = tc2r[32 * cl:32 * cl + 32].transpose([1, 0, 2])  # [j, ch, m]
            nc.sync.dma_start(
                T2[32 * cl:32 * cl + 32, :]
                .rearrange("p (ch m) -> p ch m", ch=32), src)

        # inclusive hierarchical scan over ch (4 blocks x 8) on T2
        chS = cp.tile([128, 384], F32, tag="chS")
        nc.vector.tensor_copy(chS[:], T2[:])

        def lane_views(t_ap, lanes):
            """views for compose on lane-slices of an 'lm' tile; lanes is a
            list/slice spec (lo, n, step) on the 32 lanes."""
            lo, n, step = lanes
            base = t_ap[:, 0:384].rearrange("p (lan m) -> p lan m", lan=32)
            idx = base[:, lo:lo + (n - 1) * step + 1:step, :] if step > 1 \
                else base[:, lo:lo + n, :]
            return idx  # [p, n, 12]

        def compose_lanes(dst_l, P_l, A_l, nl):
            """compose on [p, nl, 12] lane views (dims b,a,lane)."""
            def mk(v):
                pc = v[:, :, 0:9].rearrange("p n (c a) -> p n c a", c=3)

                def pcol(cc):
                    return pc[:, :, cc, :].transpose([0, 2, 1]) \
                        .unsqueeze(1).broadcast_to([128, 4, 3, nl])

                ar = v.rearrange("p n (b three) -> p n b three", b=4)

                def arow(cc):
                    return ar[:, :, :, cc].transpose([0, 2, 1]) \
                        .unsqueeze(2).broadcast_to([128, 4, 3, nl])

                def outv():
                    return v.rearrange("p n (b a) -> p b a n", b=4)

                def col3():
                    return v[:, :, 9:12]
                return pcol, arow, outv, col3

            Pp, _, _, Pc3 = mk(P_l)
            _, Aa, _, _ = mk(A_l)
            tM = lane_views(tmpM, (0, nl, 1))
            t2 = lane_views(tmp2, (0, nl, 1))
            _, _, Mo, _ = mk(tM)
            _, _, To, _ = mk(t2)
            _, _, Do, Dc3 = mk(dst_l)
            nc.vector.tensor_tensor(Mo(), Pp(0), Aa(0), AL.mult)
            nc.vector.tensor_tensor(To(), Pp(1), Aa(1), AL.mult)
            nc.vector.tensor_tensor(Mo(), Mo(), To(), AL.add)
            nc.vector.tensor_tensor(To(), Pp(2), Aa(2), AL.mult)
            nc.vector.tensor_tensor(Do(), Mo(), To(), AL.add)
            nc.vector.tensor_tensor(Dc3(), Dc3(), Pc3(), AL.add)

        for w in range(1, 8):
            # lanes ch = blk*8 + w for blk 0..3
            prev = lane_views(chS, (w - 1, 4, 8))
            curA = lane_views(T2, (w, 4, 8))
            dst = lane_views(chS, (w, 4, 8))
            compose_lanes(dst, prev, curA, 4)

        btot = cp.tile([128, 48], F32, tag="btot")
        btv = btot[:].rearrange("p (n m) -> p n m", n=4)
        nc.vector.tensor_copy(btv[:, 0:1, :], lane_views(chS, (7, 1, 1)))
        for blk in range(1, 4):
            compose_lanes(btv[:, blk:blk + 1, :], btv[:, blk - 1:blk, :],
                          lane_views(chS, (blk * 8 + 7, 1, 1)), 1)

        Pchi = cp.tile([128, 384], F32, tag="Pchi")
        nc.vector.tensor_copy(Pchi[:, 0:96], chS[:, 0:96])
        # blocks 1..3: compose btot[blk-1] (bcast over w) with chS lanes
        for blk in range(1, 4):
            bview = btv[:, blk - 1:blk, :].broadcast_to([128, 8, 12])
            compose_lanes(lane_views(Pchi, (blk * 8, 8, 1)), bview,
                          lane_views(chS, (blk * 8, 8, 1)), 8)

        Pche = cp.tile([128, 384], F32, tag="Pche")
        nc.vector.tensor_copy(Pche[:, 0:12], t_idtf[:, 0:12])
        nc.vector.tensor_copy(Pche[:, 12:384], Pchi[:, 0:372])

        # cross-block (cl) exclusive prefix of block totals via DRAM bounce
        nc.sync.dma_start(d_g[:, :], Pchi[:, 372:384])
        G4 = cp.tile([128, 48], F32, tag="G4")
        for clp in range(4):
            src = d_g.ap()[32 * clp:32 * clp + 32, :]
            src = src.unsqueeze(0).broadcast_to([4, 32, 12])
            nc.sync.dma_start(G4[:, clp * 12:(clp + 1) * 12], src)
        g4v = G4[:].rearrange("p (n m) -> p n m", n=4)
        P01t = cp.tile([128, 12], F32, tag="P01t")
        P012t = cp.tile([128, 12], F32, tag="P012t")
        compose_lanes(P01t[:].unsqueeze(1), g4v[:, 0:1, :], g4v[:, 1:2, :], 1)
        compose_lanes(P012t[:].unsqueeze(1), P01t[:].unsqueeze(1),
                      g4v[:, 2:3, :], 1)
        Pexcl = cp.tile([128, 12], F32, tag="Pexcl")
        nc.vector.tensor_copy(Pexcl[0:32, :], t_idtf[0:32, 0:12])
        nc.vector.tensor_copy(Pexcl[32:64, :], G4[32:64, 0:12])
        nc.vector.tensor_copy(Pexcl[64:96, :], P01t[64:96, :])
        nc.vector.tensor_copy(Pexcl[96:128, :], P012t[96:128, :])

        # B_chunk (in level-2 lane layout) = Pexcl o S_excl
        Bcj = cp.tile([128, 384], F32, tag="Bcj")
        compose_lanes(lane_views(Bcj, (0, 32, 1)),
                      Pexcl[:].unsqueeze(1).broadcast_to([128, 32, 12]),
                      lane_views(Pche, (0, 32, 1)), 32)
        nc.sync.dma_start(d_b2[:, :], Bcj[:])
        Bch = cp.tile([128, 384], F32, tag="Bch")
        b2r = d_b2.ap().rearrange("p (ch m) -> p ch m", ch=32)
        for cl in range(4):
            src = b2r[32 * cl:32 * cl + 32].transpose([1, 0, 2])  # [ch, j, m]
            nc.sync.dma_start(
                Bch[32 * cl:32 * cl + 32, :]
                .rearrange("p (j m) -> p j m", j=32), src)

        # C4: apply  p = B.t + B.R @ q
        qv = q_all[:].rearrange("p (k x j) -> p k x j", k=24, x=3)
        Bv = Bch[:].rearrange("p (j m) -> p j m", j=32)
        pv = p_all[:].rearrange("p (k a j) -> p k a j", k=24, a=3)
        tA = sq_all[:]  # reuse as scratch [128, 2304]
        tAv = tA.rearrange("p (k a j) -> p k a j", k=24, a=3)
        tB = ct_all[:]  # reuse as scratch
        tBv = tB.rearrange("p (k a j) -> p k a j", k=24, a=3)

        def qx(cc):
            return qv[:, :, cc, :].unsqueeze(2).broadcast_to([128, 24, 3, 32])

        def bcol(cc):
            v = Bv[:, :, 3 * cc:3 * cc + 3].transpose([0, 2, 1])  # [p,a,j]
            return v.unsqueeze(1).broadcast_to([128, 24, 3, 32])

        nc.vector.tensor_tensor(tAv, qx(0), bcol(0), AL.mult)
        nc.vector.tensor_tensor(tBv, qx(1), bcol(1), AL.mult)
        nc.vector.tensor_tensor(tAv, tAv, tBv, AL.add)
        nc.vector.tensor_tensor(tBv, qx(2), bcol(2), AL.mult)
        nc.vector.tensor_tensor(tAv, tAv, tBv, AL.add)
        nc.vector.tensor_tensor(pv, tAv, bcol(3), AL.add)
        o16 = cp.tile([128, 2304], F16, tag="o16")
        nc.vector.tensor_copy(o16[:], p_all[:])
        nc.sync.dma_start(o_scan[:, :], o16[:])

    nc.compile()
    return nc


# --------------------------------------------------------------------------
# host wrapper
# --------------------------------------------------------------------------

_NC_CACHE = []
_EXEC_CACHE = {}


def _get_nc():
    if not _NC_CACHE:
        _NC_CACHE.append(build_nc())
    return _NC_CACHE[0]


def _get_exec():
    if _EXEC_CACHE:
        return _EXEC_CACHE["fn"], _EXEC_CACHE["meta"]
    nc = _get_nc()
    import jax
    from jax.sharding import Mesh, PartitionSpec
    from jax.experimental.shard_map import shard_map
    from concourse import bass2jax
    bass2jax.install_neuronx_cc_hook()
    assert nc.dbg_addr is None

    partition_name = (nc.partition_id_tensor.name
                      if nc.partition_id_tensor else None)
    in_names, out_names, out_avals, zero_outs = [], [], [], []
    for alloc in nc.m.functions[0].allocations:
        if not isinstance(alloc, mybir.MemoryLocationSet):
            continue
        name = alloc.memorylocations[0].name
        if alloc.kind == "ExternalInput":
            if name != partition_name:
                in_names.append(name)
        elif alloc.kind == "ExternalOutput":
            shape = tuple(alloc.tensor_shape)
            dtype = mybir.dt.np(alloc.dtype)
            out_names.append(name)
            out_avals.append(jax.core.ShapedArray(shape, dtype))
            zero_outs.append(np.zeros((NCORE * shape[0], *shape[1:]), dtype))
    n_params = len(in_names)
    all_in = in_names + out_names + \
        ([partition_name] if partition_name else [])
    donate = tuple(range(n_params, n_params + len(out_names)))

    def _body(*args):
        operands = list(args)
        if partition_name is not None:
            operands.append(bass2jax.partition_id_tensor())
        outs = bass2jax._bass_exec_p.bind(
            *operands,
            out_avals=tuple(out_avals), in_names=tuple(all_in),
            out_names=tuple(out_names), lowering_input_output_aliases=(),
            sim_require_finite=True, sim_require_nnan=True, nc=nc)
        return tuple(outs)

    devices = jax.devices()[:NCORE]
    mesh = Mesh(np.asarray(devices), ("core",))
    nin = n_params + len(out_names)
    fn = jax.jit(
        shard_map(_body, mesh=mesh,
                  in_specs=(PartitionSpec("core"),) * nin,
                  out_specs=(PartitionSpec("core"),) * len(out_names),
                  check_rep=False),
        donate_argnums=donate, keep_unused=True)
    meta = dict(in_names=in_names, out_names=out_names, zero_outs=zero_outs)
    _EXEC_CACHE.update(fn=fn, meta=meta)
    return fn, meta


def _bf16_hilo(x):
    hi = x.astype(ml_dtypes.bfloat16)
    lo = (x - hi.astype(np.float32)).astype(ml_dtypes.bfloat16)
    return np.concatenate([hi, lo], axis=1)


def make_globals(inputs):
    """Build the axis-0-concatenated global arrays shard_map slices."""
    seq = np.asarray(inputs["seq"])
    kmer = np.asarray(inputs["kmer"])
    pssm = np.asarray(inputs["pssm"], dtype=np.float32)
    seq_embed = np.asarray(inputs["seq_embed"], dtype=np.float32)
    kmer_embed = np.asarray(inputs["kmer_embed"], dtype=np.float32)
    W0 = np.asarray(inputs["W0"], dtype=np.float32)
    b0 = np.asarray(inputs["b0"], dtype=np.float32)
    We = np.asarray(inputs["We"], dtype=np.float32)
    be = np.asarray(inputs["be"], dtype=np.float32)
    W1 = np.asarray(inputs["W1"], dtype=np.float32)
    b1 = np.asarray(inputs["b1"], dtype=np.float32)

    # folded kmer table, int16 fixed point
    KW = kmer_embed @ W0[16:272]                       # (10648, 64)
    q = np.rint(KW * (32767.0 / TSCALE))
    np.clip(q, -32767.0, 32767.0, out=q)
    kwt = np.zeros((NKPAD, 64), np.int16)
    kwt[:NKMER] = q.astype(np.int16)

    # folded seq table, exact bf16 hi|lo
    swp = _bf16_hilo(seq_embed @ W0[:16] + b0)         # (20, 128)

    # pssm -> biased uint16 fixed point, packed [4*21, 16*512] per core
    pq = (pssm * 65535.0 + 0.5).astype(np.int32) - 32768
    pq16 = pq.astype(np.int16).reshape(16, 4, 16, NCORE, 32, 21)
    pss = np.ascontiguousarray(pq16.transpose(3, 1, 5, 0, 2, 4)) \
        .reshape(NCORE * 84, 8192)

    # gather indices: [16, TOK/16] wrap per core
    def wrap_idx(m):
        w = m.astype(np.int16).reshape(L, NCORE, BS).transpose(1, 0, 2) \
            .reshape(NCORE, TOK // 16, 16).transpose(0, 2, 1)
        return np.ascontiguousarray(w).reshape(NCORE * 16, TOK // 16)

    identk = np.tile(np.eye(64, dtype=ml_dtypes.bfloat16), (2, 1))
    id12 = np.array([[1, 0, 0, 0, 1, 0, 0, 0, 1, 0, 0, 0]], np.float32)

    return {
        "kwt": kwt if COLL else np.tile(kwt, (NCORE, 1)),
        "swp": np.tile(swp, (NCORE, 1)),
        "pssm_q": pss,
        "kidx": wrap_idx(kmer),
        "sidx": wrap_idx(seq),
        "w0p": np.tile(np.ascontiguousarray(W0[272:293]), (NCORE, 1)),
        "wwe": np.tile(We, (NCORE, 1)),
        "ww1": np.tile(np.ascontiguousarray(W1), (NCORE, 1)),
        "becol": np.tile(be[:, None], (NCORE, 1)),
        "b1col": np.tile(b1[:, None], (NCORE, 1)),
        "identk": np.tile(identk, (NCORE, 1)),
        "id12": np.tile(id12, (NCORE, 1)),
    }


def make_in_maps(inputs):
    """Per-core input dicts (for the run_bass_kernel_spmd trace path)."""
    g = make_globals(inputs)
    maps = []
    for c in range(NCORE):
        m = {}
        for name, arr in g.items():
            P = arr.shape[0] // NCORE
            m[name] = np.ascontiguousarray(arr[c * P:(c + 1) * P])
        maps.append(m)
    return maps


def unpack_output(o):
    """(NCORE*128, 2304) fp16 global -> (3072, 256, 3) f32."""
    arr = np.asarray(o).astype(np.float32).reshape(NCORE, 128, S, 3, 32)
    return np.ascontiguousarray(arr.transpose(1, 2, 0, 4, 3)) \
        .reshape(N3, B, 3)


def kernel(**inputs):
    fn, meta = _get_exec()
    g = make_globals(inputs)
    args = [g[n] for n in meta["in_names"]] + list(meta["zero_outs"])
    outs = fn(*args)
    return unpack_output(np.asarray(outs[0]))


# revision 9
# speedup vs baseline: 5.7712x; 1.2216x over previous
"""Trainium2 Bass kernel for nn_Baseline_9904194584728.

Pipeline: embedding gathers + MLP (293->64->64->64->9) + pnerf scan.

The wall-clock of a call is dominated by the axon tunnel (~30-40 MB/s each
way), so the kernel is organized around minimizing bytes on the wire:

  * The kmer table is folded with W0 on the host (KW = kmer_embed @
    W0[16:272], 10648x64) and shipped as int16 fixed-point (scale 0.25),
    sharded 1/8 per core; an on-device AllGather rebuilds the full table,
    which is then dequantized and expanded into bf16 hi|lo pairs packed in
    128-wide rows (256B) so dma_gather's transpose mode lands them directly
    in [feature, batch] layout; a stacked [I64;I64] identity matmul
    reconstitutes hi+lo into fp32 PSUM.
  * pssm ships as uint16 fixed-point (biased int16), dequantized on the DVE.
  * Gather indices ship as [16, TOK/16] int16 and are replicated to the
    128-partition wrap layout by a broadcast DMA on device.
  * The output ships as fp16 (the pnerf scan amplifies *input* noise ~30x,
    but output quantization is post-scan and harmless).
  * A persistent jit of the shard_map'd bass_exec call avoids per-call
    retracing, and the output is fetched from device exactly once.

pnerf is algebraically an associative prefix product of rigid transforms:
R_{i+1} = R_i M_i, c_{i+1} = c_i + R_i t_i. The device scan does: pointwise
M build -> 24-step within-chunk prefix over 128 chunks (batched on
partitions) -> hierarchical chunk-carry prefix -> batched apply of boundary
transforms. Data-parallel over B across the 8 cores (B_s = 32 per core).
"""

import sys
sys.path.insert(0, "/opt/trn_rl_repo")

import os
import numpy as np
import ml_dtypes
from contextlib import ExitStack

import concourse.bass as bass
import concourse.tile as tile
from concourse import bacc, mybir

F32 = mybir.dt.float32
F16 = mybir.dt.float16
BF16 = mybir.dt.bfloat16
I16 = mybir.dt.int16
AL = mybir.AluOpType
AF = mybir.ActivationFunctionType

NCORE = 8
L = 1024
B = 256
BS = B // NCORE            # 32 batch per core
TOK = L * BS               # 32768 tokens per core
NT = TOK // 512            # 64 batch-tiles of 512
NSUP = 8                   # supertiles of 4096 tokens (gather granularity)
NKMER = 10648
NKPAD = 10752              # padded to 128*84 for the [128, 84*64] SBUF view
KSH = NKPAD // NCORE       # 1344 table rows shipped per core
N3 = 3 * L                 # 3072 chain length
S = 24                     # chunk size (level-1)
C = N3 // S                # 128 chunks
EPS2 = 1e-24
TSCALE = 0.25              # int16 table scale: value = q * TSCALE / 32767
COLL = os.environ.get("KCOLL", "1") == "1"


# --------------------------------------------------------------------------
# device kernel builder
# --------------------------------------------------------------------------

def _compose_views(t_ap, mode):
    """Return (pcol, arow, outv, col3) view factories for a [128, 384]
    transform tile.

    mode 'mj':  free = m*32 + lane   (m-major; lane = j or ch, 32 lanes)
    mode 'lm':  free = lane*12 + m   (lane-major)
    All views have dims (b, a, lane) with counts (4, 3, 32).
    """
    if mode == 'mj':
        def pcol(cc):
            v = t_ap[:, 3 * cc * 32:(3 * cc + 3) * 32]
            v = v.rearrange("p (a j) -> p a j", a=3)
            return v.unsqueeze(1).broadcast_to([128, 4, 3, 32])

        def arow(cc):
            v = t_ap[:, 0:384].rearrange("p (b three j) -> p b three j",
                                         b=4, three=3)
            v = v[:, :, cc, :]
            return v.unsqueeze(2).broadcast_to([128, 4, 3, 32])

        def outv():
            return t_ap[:, 0:384].rearrange("p (b a j) -> p b a j", b=4, a=3)

        def col3():
            return t_ap[:, 288:384]
    else:  # 'lm'
        def pcol(cc):
            v = t_ap[:, 0:384].rearrange("p (lan m) -> p lan m", lan=32)
            v = v[:, :, 3 * cc:3 * cc + 3]          # [p, lan, a]
            v = v.transpose([0, 2, 1])              # [p, a, lan]
            return v.unsqueeze(1).broadcast_to([128, 4, 3, 32])

        def arow(cc):
            v = t_ap[:, 0:384].rearrange("p (lan b three) -> p lan b three",
                                         lan=32, b=4)
            v = v[:, :, :, cc]                      # [p, lan, b]
            v = v.transpose([0, 2, 1])              # [p, b, lan]
            return v.unsqueeze(2).broadcast_to([128, 4, 3, 32])

        def outv():
            v = t_ap[:, 0:384].rearrange("p (lan b a) -> p lan b a",
                                         lan=32, b=4)
            return v.transpose([0, 2, 3, 1])        # [p, b, a, lan]

        def col3():
            v = t_ap[:, 0:384].rearrange("p (lan m) -> p lan m", lan=32)
            return v[:, :, 9:12]                    # [p, lan, a]
    return pcol, arow, outv, col3


def _emit_compose(nc, dst, P, A, tmpM, tmp2, mode):
    """dst = P o A for transform tiles [128, 384] in the given layout.
    tmpM/tmp2 are scratch [128, 384] tiles (same layout assumed; only
    used through the same view factories)."""
    Pp, _, _, Pc3 = _compose_views(P, mode)
    _, Aa, _, _ = _compose_views(A, mode)
    _, _, Mo, _ = _compose_views(tmpM, mode)
    _, _, To, _ = _compose_views(tmp2, mode)
    Dp, _, Do, Dc3 = _compose_views(dst, mode)
    nc.vector.tensor_tensor(Mo(), Pp(0), Aa(0), AL.mult)
    nc.vector.tensor_tensor(To(), Pp(1), Aa(1), AL.mult)
    nc.vector.tensor_tensor(tmpM[:, 0:384], tmpM[:, 0:384], tmp2[:, 0:384],
                            AL.add)
    nc.vector.tensor_tensor(To(), Pp(2), Aa(2), AL.mult)
    nc.vector.tensor_tensor(dst[:, 0:384], tmpM[:, 0:384], tmp2[:, 0:384],
                            AL.add)
    # translation: dst.t += P.t
    nc.vector.tensor_tensor(Dc3(), Dc3(), Pc3(), AL.add)


def build_nc():
    nc = bacc.Bacc("TRN2", target_bir_lowering=False, debug=False,
                   num_devices=NCORE)

    # ---------------- I/O ----------------
    kw_rows = KSH if COLL else NKPAD
    d_kwt = nc.declare_dram_parameter("kwt", [kw_rows, 64], I16, isOutput=False)
    d_swp = nc.declare_dram_parameter("swp", [20, 128], BF16, isOutput=False)
    d_w0p = nc.declare_dram_parameter("w0p", [21, 64], F32, isOutput=False)
    d_we = nc.declare_dram_parameter("wwe", [64, 64], F32, isOutput=False)
    d_w1 = nc.declare_dram_parameter("ww1", [64, 9], F32, isOutput=False)
    d_be = nc.declare_dram_parameter("becol", [64, 1], F32, isOutput=False)
    d_b1 = nc.declare_dram_parameter("b1col", [9, 1], F32, isOutput=False)
    d_idk = nc.declare_dram_parameter("identk", [128, 64], BF16, isOutput=False)
    d_id12 = nc.declare_dram_parameter("id12", [1, 12], F32, isOutput=False)
    d_kidx = nc.declare_dram_parameter("kidx", [16, TOK // 16], I16,
                                       isOutput=False)
    d_sidx = nc.declare_dram_parameter("sidx", [16, TOK // 16], I16,
                                       isOutput=False)
    d_pssm = nc.declare_dram_parameter("pssm_q", [84, 8192], I16,
                                       isOutput=False)
    o_scan = nc.declare_dram_parameter("o_scan", [128, 2304], F16,
                                       isOutput=True)

    # ---------------- internal DRAM ----------------
    kwp = nc.dram_tensor("kwp", [NKPAD, 128], BF16)
    srf_d = nc.dram_tensor("srf_d", [9, TOK], F32)
    d_tc2 = nc.dram_tensor("d_tc2", [128, 384], F32)
    d_g = nc.dram_tensor("d_g", [128, 12], F32)
    d_b2 = nc.dram_tensor("d_b2", [128, 384], F32)
    if COLL:
        cc_in = nc.dram_tensor("cc_in", [KSH, 64], I16)
        cc_out = nc.dram_tensor("cc_out", [NKPAD, 64], I16,
                                addr_space="Shared")

    with ExitStack() as ctx:
        tc = ctx.enter_context(tile.TileContext(nc))

        # persistent pool
        pw = ctx.enter_context(tc.tile_pool(name="pw", bufs=1))
        t_w0p4 = pw.tile([128, 64], F32, tag="w0p4")
        t_we = pw.tile([64, 64], F32, tag="we")
        t_w1 = pw.tile([64, 9], F32, tag="w1")
        t_be = pw.tile([64, 1], F32, tag="be")
        t_b1 = pw.tile([9, 1], F32, tag="b1")
        t_idk = pw.tile([128, 64], BF16, tag="idk")
        t_kidx = pw.tile([128, TOK // 16], I16, tag="kidx")
        t_sidx = pw.tile([128, TOK // 16], I16, tag="sidx")
        t_pssm = pw.tile([128, 8192], F32, tag="pssm")
        t_idtf = pw.tile([128, 12], F32, tag="idtf")

        for q in range(4):
            nc.sync.dma_start(t_w0p4[32 * q:32 * q + 21, :], d_w0p[:, :])
        nc.sync.dma_start(t_we[:], d_we[:, :])
        nc.sync.dma_start(t_w1[:], d_w1[:, :])
        nc.sync.dma_start(t_be[:], d_be[:, :])
        nc.sync.dma_start(t_b1[:], d_b1[:, :])
        nc.sync.dma_start(t_idk[:], d_idk[:, :])
        nc.sync.dma_start(t_idtf[:], d_id12[0:1, :].broadcast_to([128, 12]))
        # wrap-replicate the gather indices 16 -> 128 partitions
        for g in range(8):
            nc.sync.dma_start(t_kidx[16 * g:16 * g + 16, :], d_kidx[:, :])
            nc.sync.dma_start(t_sidx[16 * g:16 * g + 16, :], d_sidx[:, :])

        # ---------------- phase A: table expand + pssm dequant ----------
        with ExitStack() as actx:
            ap_ = actx.enter_context(tc.tile_pool(name="ap", bufs=1))

            # pssm: biased-int16 -> fp32 in [0, 1]
            t_praw = ap_.tile([128, 8192], I16, tag="praw")
            for q in range(4):
                nc.sync.dma_start(t_praw[32 * q:32 * q + 21, :],
                                  d_pssm[21 * q:21 * q + 21, :])
                nc.vector.tensor_scalar(
                    t_pssm[32 * q:32 * q + 21, :],
                    t_praw[32 * q:32 * q + 21, :],
                    32768.5, 1.0 / 65536.0, AL.add, AL.mult)

            # kmer table: int16 shard -> (AllGather) -> bf16 hi|lo rows
            if COLL:
                nc.sync.dma_start(cc_in[:, :], d_kwt[:, :])
                nc.gpsimd.collective_compute(
                    "AllGather", AL.bypass,
                    replica_groups=[list(range(NCORE))],
                    ins=[cc_in.ap().opt()],
                    outs=[cc_out.ap().opt()])
                src_tbl = cc_out.ap()
            else:
                src_tbl = d_kwt[:, :]
            t_raw = ap_.tile([128, 5376], I16, tag="traw")
            nc.sync.dma_start(
                t_raw[:].rearrange("p (q c) -> p q c", q=84),
                src_tbl.rearrange("(p q) c -> p q c", p=128))
            t_tf = ap_.tile([128, 5376], F32, tag="ttf")
            nc.vector.tensor_scalar_mul(t_tf[:], t_raw[:], TSCALE / 32767.0)
            t_hi = ap_.tile([128, 5376], BF16, tag="thi")
            nc.scalar.activation(t_hi[:], t_tf[:], AF.Copy)
            t_lo = ap_.tile([128, 5376], BF16, tag="tlo")
            nc.vector.tensor_tensor(t_lo[:], t_tf[:], t_hi[:], AL.subtract)
            kwv = kwp.ap().rearrange("(p q) (h c) -> p h q c", p=128, h=2)
            nc.sync.dma_start(kwv[:, 0],
                              t_hi[:].rearrange("p (q c) -> p q c", q=84))
            nc.sync.dma_start(kwv[:, 1],
                              t_lo[:].rearrange("p (q c) -> p q c", q=84))

        # ---------------- phase B: MLP ----------------
        with ExitStack() as bctx:
            gp = bctx.enter_context(tc.tile_pool(name="gp", bufs=2))
            hb = bctx.enter_context(tc.tile_pool(name="hb", bufs=3))
            bps = bctx.enter_context(
                tc.tile_pool(name="bps", bufs=3, space="PSUM"))
            sps = bctx.enter_context(
                tc.tile_pool(name="sps", bufs=2, space="PSUM"))
            sf = bctx.enter_context(tc.tile_pool(name="sf", bufs=2))

            GW = TOK // NSUP                     # 4096 idx per gather
            for sup in range(NSUP):
                kg = gp.tile([128, GW], BF16, tag="kg")
                sg = gp.tile([128, GW], BF16, tag="sg")
                isl = slice(sup * (GW // 16), (sup + 1) * (GW // 16))
                nc.gpsimd.dma_gather(
                    kg[:].rearrange("p (one n) -> p one n", one=1),
                    kwp[:, :], t_kidx[:, isl], num_idxs=GW, num_idxs_reg=GW,
                    elem_size=128, transpose=True, single_packet=False)
                nc.gpsimd.dma_gather(
                    sg[:].rearrange("p (one n) -> p one n", one=1),
                    d_swp[:, :], t_sidx[:, isl], num_idxs=GW, num_idxs_reg=GW,
                    elem_size=128, transpose=True, single_packet=False)
                srfS = sf.tile([9, GW], F32, tag="srfS")
                for tp in range(NT // NSUP):     # 8 batch-tiles per supertile
                    t = sup * (NT // NSUP) + tp
                    q, r = t % 4, t // 4
                    csl = slice(tp * 512, (tp + 1) * 512)
                    ph0 = bps.tile([64, 512], F32, tag="ph")
                    nc.tensor.matmul(ph0[:], t_idk[:], kg[:, csl],
                                     start=True, stop=False)
                    nc.tensor.matmul(ph0[:], t_idk[:], sg[:, csl],
                                     start=False, stop=False)
                    nc.tensor.matmul(
                        ph0[:], t_w0p4[32 * q:32 * q + 21, :],
                        t_pssm[32 * q:32 * q + 21, 512 * r:512 * r + 512],
                        start=False, stop=True,
                        tile_position=(32 * q, 0))
                    h0 = hb.tile([64, 512], F32, tag="h0")
                    nc.scalar.activation(h0[:], ph0[:], AF.Copy)
                    ph1 = bps.tile([64, 512], F32, tag="ph")
                    nc.tensor.matmul(ph1[:], t_we[:], h0[:], start=True,
                                     stop=True)
                    h1 = hb.tile([64, 512], F32, tag="h1")
                    nc.vector.tensor_scalar(h1[:], ph1[:], t_be[:], 0.0,
                                            AL.add, AL.max)
                    ph2 = bps.tile([64, 512], F32, tag="ph")
                    nc.tensor.matmul(ph2[:], t_we[:], h1[:], start=True,
                                     stop=True)
                    h2 = hb.tile([64, 512], F32, tag="h2")
                    nc.scalar.activation(h2[:], ph2[:], AF.Relu, bias=t_be[:],
                                         scale=1.0)
                    ps3 = sps.tile([9, 512], F32, tag="ps3")
                    nc.tensor.matmul(ps3[:], t_w1[:], h2[:], start=True,
                                     stop=True)
                    nc.vector.tensor_scalar(srfS[:, csl], ps3[:], t_b1[:],
                                            None, AL.add)
                nc.sync.dma_start(srf_d[:, sup * GW:(sup + 1) * GW], srfS[:])

        # ---------------- phase C: scan ----------------
        cp = ctx.enter_context(tc.tile_pool(name="cp", bufs=1))
        ct_all = cp.tile([128, 2304], F32, tag="ct")
        A_all = cp.tile([128, 24 * 384], F32, tag="Aall")
        q_all = cp.tile([128, 2304], F32, tag="qall")
        p_all = cp.tile([128, 2304], F32, tag="pall")
        sq_all = cp.tile([128, 2304], F32, tag="sqall")
        tmp768a = cp.tile([128, 768], F32, tag="t768a")
        tmp768b = cp.tile([128, 768], F32, tag="t768b")
        n2t = cp.tile([128, 768], F32, tag="n2")
        n2ct = cp.tile([128, 768], F32, tag="n2c")
        rnt = cp.tile([128, 768], F32, tag="rn")
        rnct = cp.tile([128, 768], F32, tag="rnc")

        # C0: permute srf -> ct_all [c, (k*3+x)*32 + j]
        srf_r = srf_d.ap().rearrange("(r x) (c k1 j) -> r x c k1 j",
                                     r=3, x=3, c=128, k1=8)
        ct_r = ct_all[:].rearrange("p (k1 k2 x j) -> p k1 k2 x j",
                                   k1=8, k2=3, x=3)
        for k2 in range(3):
            for x in range(3):
                src = srf_r[k2, x]                       # [c, k1, j]
                nc.sync.dma_start(ct_r[:, :, k2, x, :], src)

        # C1: pointwise transform build
        ctv4 = ct_all[:].rearrange("p (k x j) -> p k x j", k=24, x=3)
        sqv4 = sq_all[:].rearrange("p (k x j) -> p k j x", k=24, x=3)
        Af = A_all[:].rearrange("p (k m j) -> p k m j", k=24, m=12)
        n2v = n2t[:].rearrange("p (k j) -> p k j", k=24)
        n2cv = n2ct[:].rearrange("p (k j) -> p k j", k=24)
        rnv3 = rnt[:].rearrange("p (k j) -> p k j", k=24).unsqueeze(2) \
                     .broadcast_to([128, 24, 3, 32])
        rncv = rnct[:].rearrange("p (k j) -> p k j", k=24)

        def ctx_(x):
            return ctv4[:, :, x, :]

        nc.scalar.activation(sq_all[:], ct_all[:], AF.Square)
        nc.vector.tensor_reduce(n2v.unsqueeze(-1), sqv4, mybir.AxisListType.X,
                                AL.add)
        nc.vector.tensor_reduce(n2cv.unsqueeze(-1), sqv4[:, :, :, 1:3],
                                mybir.AxisListType.X, AL.add)
        nc.vector.tensor_scalar_max(n2t[:], n2t[:], EPS2)
        nc.vector.tensor_scalar_max(n2ct[:], n2ct[:], EPS2)
        nc.scalar.activation(tmp768a[:], n2t[:], AF.Sqrt)
        nc.scalar.activation(tmp768b[:], n2ct[:], AF.Sqrt)
        nc.vector.reciprocal_approx_accurate(rnt[:], tmp768a[:], sq_all[:, 0:768])
        nc.vector.reciprocal_approx_accurate(rnct[:], tmp768b[:],
                                             sq_all[:, 768:1536])
        # A columns: c0 = ct*rn ; t = ct ; c2 = (0, -z*rnc, y*rnc)
        nc.vector.tensor_tensor(Af[:, :, 0:3, :], ctv4, rnv3, AL.mult)
        nc.scalar.activation(Af[:, :, 9:12, :], ctv4, AF.Copy)
        nc.vector.tensor_scalar_mul(Af[:, :, 6, :], ctx_(0), 0.0)
        nc.vector.scalar_tensor_tensor(Af[:, :, 7, :], ctx_(2), -1.0, rncv,
                                       AL.mult, AL.mult)
        nc.vector.tensor_tensor(Af[:, :, 8, :], ctx_(1), rncv, AL.mult)
        # c1 = n^ x c0^
        nc.vector.tensor_tensor(Af[:, :, 3, :], Af[:, :, 7, :],
                                Af[:, :, 2, :], AL.mult)
        nc.vector.tensor_tensor(tmp768a[:].rearrange("p (k j) -> p k j", k=24),
                                Af[:, :, 8, :], Af[:, :, 1, :], AL.mult)
        nc.vector.tensor_tensor(Af[:, :, 3, :], Af[:, :, 3, :],
                                tmp768a[:].rearrange("p (k j) -> p k j", k=24),
                                AL.subtract)
        nc.vector.tensor_tensor(Af[:, :, 4, :], Af[:, :, 8, :],
                                Af[:, :, 0, :], AL.mult)
        nc.vector.scalar_tensor_tensor(Af[:, :, 5, :], Af[:, :, 7, :], -1.0,
                                       Af[:, :, 0, :], AL.mult, AL.mult)

        # C2: level-1 scan (23 steps over k)
        Pa = cp.tile([128, 384], F32, tag="Pa")
        Pb = cp.tile([128, 384], F32, tag="Pb")
        tmpM = cp.tile([128, 384], F32, tag="tmpM")
        tmp2 = cp.tile([128, 384], F32, tag="tmp2")
        nc.scalar.activation(Pa[:], A_all[:, 0:384], AF.Copy)
        nc.scalar.activation(q_all[:, 0:96], A_all[:, 288:384], AF.Copy)
        cur, nxt = Pa, Pb
        for k in range(1, S):
            Ak = A_all[:, k * 384:(k + 1) * 384]
            _emit_compose(nc, nxt, cur, Ak, tmpM, tmp2, 'mj')
            nc.scalar.activation(q_all[:, k * 96:(k + 1) * 96],
                                 nxt[:, 288:384], AF.Copy)
            cur, nxt = nxt, cur
        Pfin = cur

        # C3: level-2 (chunk-carry exclusive prefix)
        # chunk c = 32*cl + ch; level-2 lanes: partition p = j + 32*cl,
        # free lanes ch (32), so all partition slices stay contiguous.
        # repack [c, m*32+j] -> [c, j*12+m] and bounce
        Palt = cp.tile([128, 384], F32, tag="Palt")
        nc.vector.tensor_copy(
            Palt[:].rearrange("p (j m) -> p j m", j=32),
            Pfin[:].rearrange("p (m j) -> p m j", m=12).transpose([0, 2, 1]))
        nc.sync.dma_start(d_tc2[:, :], Palt[:])
        T2 = cp.tile([128, 384], F32, tag="T2")
        tc2r = d_tc2.ap().rearrange("c (j m) -> c j m", j=32)
        for cl in range(4):
            # dst partitions j (block cl) <- rows c = 32*cl + ch
            src = tc2r[32 * cl:32 * cl + 32].transpose([1, 0, 2])  # [j, ch, m]
            nc.sync.dma_start(
                T2[32 * cl:32 * cl + 32, :]
                .rearrange("p (ch m) -> p ch m", ch=32), src)

        # inclusive hierarchical scan over ch (4 blocks x 8) on T2
        chS = cp.tile([128, 384], F32, tag="chS")
        nc.vector.tensor_copy(chS[:], T2[:])

        def lane_views(t_ap, lanes):
            """views for compose on lane-slices of an 'lm' tile; lanes is a
            list/slice spec (lo, n, step) on the 32 lanes."""
            lo, n, step = lanes
            base = t_ap[:, 0:384].rearrange("p (lan m) -> p lan m", lan=32)
            idx = base[:, lo:lo + (n - 1) * step + 1:step, :] if step > 1 \
                else base[:, lo:lo + n, :]
            return idx  # [p, n, 12]

        def compose_lanes(dst_l, P_l, A_l, nl):
            """compose on [p, nl, 12] lane views (dims b,a,lane)."""
            def mk(v):
                pc = v[:, :, 0:9].rearrange("p n (c a) -> p n c a", c=3)

                def pcol(cc):
                    return pc[:, :, cc, :].transpose([0, 2, 1]) \
                        .unsqueeze(1).broadcast_to([128, 4, 3, nl])

                ar = v.rearrange("p n (b three) -> p n b three", b=4)

                def arow(cc):
                    return ar[:, :, :, cc].transpose([0, 2, 1]) \
                        .unsqueeze(2).broadcast_to([128, 4, 3, nl])

                def outv():
                    return v.rearrange("p n (b a) -> p b a n", b=4)

                def col3():
                    return v[:, :, 9:12]
                return pcol, arow, outv, col3

            Pp, _, _, Pc3 = mk(P_l)
            _, Aa, _, _ = mk(A_l)
            tM = lane_views(tmpM, (0, nl, 1))
            t2 = lane_views(tmp2, (0, nl, 1))
            _, _, Mo, _ = mk(tM)
            _, _, To, _ = mk(t2)
            _, _, Do, Dc3 = mk(dst_l)
            nc.vector.tensor_tensor(Mo(), Pp(0), Aa(0), AL.mult)
            nc.vector.tensor_tensor(To(), Pp(1), Aa(1), AL.mult)
            nc.vector.tensor_tensor(Mo(), Mo(), To(), AL.add)
            nc.vector.tensor_tensor(To(), Pp(2), Aa(2), AL.mult)
            nc.vector.tensor_tensor(Do(), Mo(), To(), AL.add)
            nc.vector.tensor_tensor(Dc3(), Dc3(), Pc3(), AL.add)

        for w in range(1, 8):
            # lanes ch = blk*8 + w for blk 0..3
            prev = lane_views(chS, (w - 1, 4, 8))
            curA = lane_views(T2, (w, 4, 8))
            dst = lane_views(chS, (w, 4, 8))
            compose_lanes(dst, prev, curA, 4)

        btot = cp.tile([128, 48], F32, tag="btot")
        btv = btot[:].rearrange("p (n m) -> p n m", n=4)
        nc.vector.tensor_copy(btv[:, 0:1, :], lane_views(chS, (7, 1, 1)))
        for blk in range(1, 4):
            compose_lanes(btv[:, blk:blk + 1, :], btv[:, blk - 1:blk, :],
                          lane_views(chS, (blk * 8 + 7, 1, 1)), 1)

        Pchi = cp.tile([128, 384], F32, tag="Pchi")
        nc.vector.tensor_copy(Pchi[:, 0:96], chS[:, 0:96])
        # blocks 1..3: compose btot[blk-1] (bcast over w) with chS lanes
        for blk in range(1, 4):
            bview = btv[:, blk - 1:blk, :].broadcast_to([128, 8, 12])
            compose_lanes(lane_views(Pchi, (blk * 8, 8, 1)), bview,
                          lane_views(chS, (blk * 8, 8, 1)), 8)

        Pche = cp.tile([128, 384], F32, tag="Pche")
        nc.vector.tensor_copy(Pche[:, 0:12], t_idtf[:, 0:12])
        nc.vector.tensor_copy(Pche[:, 12:384], Pchi[:, 0:372])

        # cross-block (cl) exclusive prefix of block totals via DRAM bounce
        nc.sync.dma_start(d_g[:, :], Pchi[:, 372:384])
        G4 = cp.tile([128, 48], F32, tag="G4")
        for clp in range(4):
            src = d_g.ap()[32 * clp:32 * clp + 32, :]
            src = src.unsqueeze(0).broadcast_to([4, 32, 12])
            nc.sync.dma_start(G4[:, clp * 12:(clp + 1) * 12], src)
        g4v = G4[:].rearrange("p (n m) -> p n m", n=4)
        P01t = cp.tile([128, 12], F32, tag="P01t")
        P012t = cp.tile([128, 12], F32, tag="P012t")
        compose_lanes(P01t[:].unsqueeze(1), g4v[:, 0:1, :], g4v[:, 1:2, :], 1)
        compose_lanes(P012t[:].unsqueeze(1), P01t[:].unsqueeze(1),
                      g4v[:, 2:3, :], 1)
        Pexcl = cp.tile([128, 12], F32, tag="Pexcl")
        nc.vector.tensor_copy(Pexcl[0:32, :], t_idtf[0:32, 0:12])
        nc.vector.tensor_copy(Pexcl[32:64, :], G4[32:64, 0:12])
        nc.vector.tensor_copy(Pexcl[64:96, :], P01t[64:96, :])
        nc.vector.tensor_copy(Pexcl[96:128, :], P012t[96:128, :])

        # B_chunk (in level-2 lane layout) = Pexcl o S_excl
        Bcj = cp.tile([128, 384], F32, tag="Bcj")
        compose_lanes(lane_views(Bcj, (0, 32, 1)),
                      Pexcl[:].unsqueeze(1).broadcast_to([128, 32, 12]),
                      lane_views(Pche, (0, 32, 1)), 32)
        nc.sync.dma_start(d_b2[:, :], Bcj[:])
        Bch = cp.tile([128, 384], F32, tag="Bch")
        b2r = d_b2.ap().rearrange("p (ch m) -> p ch m", ch=32)
        for cl in range(4):
            src = b2r[32 * cl:32 * cl + 32].transpose([1, 0, 2])  # [ch, j, m]
            nc.sync.dma_start(
                Bch[32 * cl:32 * cl + 32, :]
                .rearrange("p (j m) -> p j m", j=32), src)

        # C4: apply  p = B.t + B.R @ q
        qv = q_all[:].rearrange("p (k x j) -> p k x j", k=24, x=3)
        Bv = Bch[:].rearrange("p (j m) -> p j m", j=32)
        pv = p_all[:].rearrange("p (k a j) -> p k a j", k=24, a=3)
        tA = sq_all[:]  # reuse as scratch [128, 2304]
        tAv = tA.rearrange("p (k a j) -> p k a j", k=24, a=3)
        tB = ct_all[:]  # reuse as scratch
        tBv = tB.rearrange("p (k a j) -> p k a j", k=24, a=3)

        def qx(cc):
            return qv[:, :, cc, :].unsqueeze(2).broadcast_to([128, 24, 3, 32])

        def bcol(cc):
            v = Bv[:, :, 3 * cc:3 * cc + 3].transpose([0, 2, 1])  # [p,a,j]
            return v.unsqueeze(1).broadcast_to([128, 24, 3, 32])

        nc.vector.tensor_tensor(tAv, qx(0), bcol(0), AL.mult)
        nc.vector.tensor_tensor(tBv, qx(1), bcol(1), AL.mult)
        nc.vector.tensor_tensor(tAv, tAv, tBv, AL.add)
        nc.vector.tensor_tensor(tBv, qx(2), bcol(2), AL.mult)
        nc.vector.tensor_tensor(tAv, tAv, tBv, AL.add)
        nc.vector.tensor_tensor(pv, tAv, bcol(3), AL.add)
        o16 = cp.tile([128, 2304], F16, tag="o16")
        nc.vector.tensor_copy(o16[:], p_all[:])
        nc.sync.dma_start(o_scan[:, :], o16[:])

    nc.compile()
    return nc


# --------------------------------------------------------------------------
# host wrapper
# --------------------------------------------------------------------------

_NC_CACHE = []
_EXEC_CACHE = {}


def _get_nc():
    if not _NC_CACHE:
        _NC_CACHE.append(build_nc())
    return _NC_CACHE[0]


def _get_exec():
    if _EXEC_CACHE:
        return _EXEC_CACHE["fn"], _EXEC_CACHE["meta"]
    nc = _get_nc()
    import jax
    from jax.sharding import Mesh, PartitionSpec
    from jax.experimental.shard_map import shard_map
    from concourse import bass2jax
    bass2jax.install_neuronx_cc_hook()
    assert nc.dbg_addr is None

    partition_name = (nc.partition_id_tensor.name
                      if nc.partition_id_tensor else None)
    in_names, out_names, out_avals, zero_outs = [], [], [], []
    for alloc in nc.m.functions[0].allocations:
        if not isinstance(alloc, mybir.MemoryLocationSet):
            continue
        name = alloc.memorylocations[0].name
        if alloc.kind == "ExternalInput":
            if name != partition_name:
                in_names.append(name)
        elif alloc.kind == "ExternalOutput":
            shape = tuple(alloc.tensor_shape)
            dtype = mybir.dt.np(alloc.dtype)
            out_names.append(name)
            out_avals.append(jax.core.ShapedArray(shape, dtype))
            zero_outs.append(np.zeros((NCORE * shape[0], *shape[1:]), dtype))
    n_params = len(in_names)
    all_in = in_names + out_names + \
        ([partition_name] if partition_name else [])
    donate = tuple(range(n_params, n_params + len(out_names)))

    def _body(*args):
        operands = list(args)
        if partition_name is not None:
            operands.append(bass2jax.partition_id_tensor())
        outs = bass2jax._bass_exec_p.bind(
            *operands,
            out_avals=tuple(out_avals), in_names=tuple(all_in),
            out_names=tuple(out_names), lowering_input_output_aliases=(),
            sim_require_finite=True, sim_require_nnan=True, nc=nc)
        return tuple(outs)

    devices = jax.devices()[:NCORE]
    mesh = Mesh(np.asarray(devices), ("core",))
    nin = n_params + len(out_names)
    fn = jax.jit(
        shard_map(_body, mesh=mesh,
                  in_specs=(PartitionSpec("core"),) * nin,
                  out_specs=(PartitionSpec("core"),) * len(out_names),
                  check_rep=False),
        donate_argnums=donate, keep_unused=True)

    # donated output buffers are zero-filled ON DEVICE (no H2D bytes)
    import jax.numpy as jnp
    from jax.sharding import NamedSharding
    zspecs = [(z.shape, z.dtype) for z in zero_outs]
    shardings = tuple(NamedSharding(mesh, PartitionSpec("core"))
                      for _ in zspecs)
    zeros_fn = jax.jit(
        lambda: tuple(jnp.zeros(s, d) for s, d in zspecs),
        out_shardings=shardings)
    meta = dict(in_names=in_names, out_names=out_names, zero_outs=zero_outs,
                zeros_fn=zeros_fn)
    _EXEC_CACHE.update(fn=fn, meta=meta)
    return fn, meta


def _bf16_hilo(x):
    hi = x.astype(ml_dtypes.bfloat16)
    lo = (x - hi.astype(np.float32)).astype(ml_dtypes.bfloat16)
    return np.concatenate([hi, lo], axis=1)


def make_globals(inputs):
    """Build the axis-0-concatenated global arrays shard_map slices."""
    seq = np.asarray(inputs["seq"])
    kmer = np.asarray(inputs["kmer"])
    pssm = np.asarray(inputs["pssm"], dtype=np.float32)
    seq_embed = np.asarray(inputs["seq_embed"], dtype=np.float32)
    kmer_embed = np.asarray(inputs["kmer_embed"], dtype=np.float32)
    W0 = np.asarray(inputs["W0"], dtype=np.float32)
    b0 = np.asarray(inputs["b0"], dtype=np.float32)
    We = np.asarray(inputs["We"], dtype=np.float32)
    be = np.asarray(inputs["be"], dtype=np.float32)
    W1 = np.asarray(inputs["W1"], dtype=np.float32)
    b1 = np.asarray(inputs["b1"], dtype=np.float32)

    # folded kmer table, int16 fixed point. Exact round-to-nearest-even of
    # KW*32767/TSCALE via fp32 mantissa bits: for y = 1.5 + z with
    # |z| < 2^-7, bits(y) = 0x3FC00000 + round(z * 2^23).
    KW = kmer_embed @ W0[16:272]                       # (10648, 64)
    np.clip(KW, -TSCALE * 0.999, TSCALE * 0.999, out=KW)
    y = KW * ((32767.0 / TSCALE) / 8388608.0) + 1.5
    kwt = np.zeros((NKPAD, 64), np.int16)
    kwt[:NKMER] = (y.view(np.int32) - 0x3FC00000).astype(np.int16)

    # folded seq table, exact bf16 hi|lo
    swp = _bf16_hilo(seq_embed @ W0[:16] + b0)         # (20, 128)

    # pssm -> biased uint16 fixed point, packed [4*21, 16*512] per core.
    # q = floor(p * 65536) from the top mantissa bits of 1.0 + p; the bias
    # flip (^0x8000) makes the biased uint16 readable as int16; device
    # dequantizes to the bucket midpoint (q + 0.5) / 65536. The clamp keeps
    # fl(1 + p) below 2.0 so the mantissa never wraps.
    yp = np.minimum(pssm, np.float32(1.0 - 2.0 ** -16)) + 1.0
    pq16 = ((yp.view(np.int32) >> 7) ^ 0x8000).astype(np.int16) \
        .reshape(16, 4, 16, NCORE, 32, 21)
    pss = np.ascontiguousarray(pq16.transpose(3, 1, 5, 0, 2, 4)) \
        .reshape(NCORE * 84, 8192)

    # gather indices: [16, TOK/16] wrap per core
    def wrap_idx(m):
        w = m.astype(np.int16).reshape(L, NCORE, BS).transpose(1, 0, 2) \
            .reshape(NCORE, TOK // 16, 16).transpose(0, 2, 1)
        return np.ascontiguousarray(w).reshape(NCORE * 16, TOK // 16)

    identk = np.tile(np.eye(64, dtype=ml_dtypes.bfloat16), (2, 1))
    id12 = np.array([[1, 0, 0, 0, 1, 0, 0, 0, 1, 0, 0, 0]], np.float32)

    return {
        "kwt": kwt if COLL else np.tile(kwt, (NCORE, 1)),
        "swp": np.tile(swp, (NCORE, 1)),
        "pssm_q": pss,
        "kidx": wrap_idx(kmer),
        "sidx": wrap_idx(seq),
        "w0p": np.tile(np.ascontiguousarray(W0[272:293]), (NCORE, 1)),
        "wwe": np.tile(We, (NCORE, 1)),
        "ww1": np.tile(np.ascontiguousarray(W1), (NCORE, 1)),
        "becol": np.tile(be[:, None], (NCORE, 1)),
        "b1col": np.tile(b1[:, None], (NCORE, 1)),
        "identk": np.tile(identk, (NCORE, 1)),
        "id12": np.tile(id12, (NCORE, 1)),
    }


def make_in_maps(inputs):
    """Per-core input dicts (for the run_bass_kernel_spmd trace path)."""
    g = make_globals(inputs)
    maps = []
    for c in range(NCORE):
        m = {}
        for name, arr in g.items():
            P = arr.shape[0] // NCORE
            m[name] = np.ascontiguousarray(arr[c * P:(c + 1) * P])
        maps.append(m)
    return maps


def unpack_output(o):
    """(NCORE*128, 2304) fp16 global -> (3072, 256, 3) f32."""
    arr = np.asarray(o).astype(np.float32).reshape(NCORE, 128, S, 3, 32)
    return np.ascontiguousarray(arr.transpose(1, 2, 0, 4, 3)) \
        .reshape(N3, B, 3)


def kernel(**inputs):
    fn, meta = _get_exec()
    g = make_globals(inputs)
    args = [g[n] for n in meta["in_names"]] + list(meta["zeros_fn"]())
    outs = fn(*args)
    return unpack_output(np.asarray(outs[0]))
